# revision 11
# baseline (speedup 1.0000x reference)
"""Trainium2 Bass kernel for KVAdapterInjector (Qwen3-style GQA attention with
LoRA-adapted virtual KV prefix).

Sharding: tensor-parallel over heads across 8 cores. Core m gets KV head m and
Q heads 4m..4m+3. Wq/Wk/Wv sharded on output dim, Wo on input dim; partial
outputs (bf16) summed on host.

v2 design notes (cost-model driven):
- All heavy matmuls in bf16 (1.0 cycles/row, immune to the fp32r ap<256
  penalty). PSUM accumulation stays fp32. Measured end-to-end bf16 error
  ~5.5e-3 (budget 2e-2). fp8 was measured at 2.7-5e-2 per stage: rejected.
- PE-row accounting puts the tensor engine at ~370us; all other engines are
  kept under ~150us: softmax denominators stay as ones-matmuls on PE, but
  rms-norm sum/broadcast use gpsimd partition_all_reduce/broadcast (Pool),
  rsqrt = exp(-0.5*ln(x)) on Act (single activation table: ln+exp+square),
  mask-adds and PSUM drains ride Pool, rope elementwise rides DVE in bf16
  (2x mode).
- Causal diagonal blocks are trimmed: block j of a 512-query chunk only
  computes queries >= 128*j, with a constant [128,128] triangular mask tile.
- Chunk-pipelined: proj(c) -> norm/rope(c) -> attention(c) -> outproj(c),
  with PSUM pools sized to exactly 8 banks so phases from adjacent chunks
  overlap across engines.
"""
import sys

sys.path.insert(0, "/opt/trn_rl_repo")

import numpy as np
import ml_dtypes

import concourse.bass as bass
import concourse.mybir as mybir
import concourse.tile as tile
from concourse import bacc
from concourse import bass_isa
from concourse.bass_utils import run_bass_kernel_spmd

F32 = mybir.dt.float32
F32R = mybir.dt.float32r
BF16 = mybir.dt.bfloat16
AX = mybir.AxisListType
ALU = mybir.AluOpType
ACTF = mybir.ActivationFunctionType
RED = bass_isa.ReduceOp

T = 2048
D = 4096
HD = 128
NQH = 4          # q heads per core
R = 64           # virtual tokens
RANK = 16
EPS = 1e-6
SCALING = HD ** -0.5
NTC = 4          # T chunks of 512
TC = 512
ND = D // 128    # 32 contraction tiles
NKB = T // 128   # 16 key blocks (real)


def build_nc():
    nc = bacc.Bacc(None, target_bir_lowering=False, debug=False)

    # ---- DRAM I/O (bf16 activations/weights prepared on host) ----
    hsT = nc.dram_tensor("hsT", (D, T), BF16, kind="ExternalInput")
    wq = nc.dram_tensor("wq", (D, NQH * HD), BF16, kind="ExternalInput")
    wk = nc.dram_tensor("wk", (D, HD), BF16, kind="ExternalInput")
    wv = nc.dram_tensor("wv", (D, HD), BF16, kind="ExternalInput")
    wo = nc.dram_tensor("wo", (NQH * HD, D), BF16, kind="ExternalInput")
    cwq = nc.dram_tensor("cwq", (HD, T), BF16, kind="ExternalInput")
    swq = nc.dram_tensor("swq", (HD, T), BF16, kind="ExternalInput")
    cwk = nc.dram_tensor("cwk", (HD, T), BF16, kind="ExternalInput")
    swk = nc.dram_tensor("swk", (HD, T), BF16, kind="ExternalInput")
    masktri = nc.dram_tensor("masktri", (128, 128), F32, kind="ExternalInput")
    vkT = nc.dram_tensor("vkT", (HD, R), F32, kind="ExternalInput")
    vvT = nc.dram_tensor("vvT", (HD, R), F32, kind="ExternalInput")
    lkA = nc.dram_tensor("lkA", (HD, RANK), F32, kind="ExternalInput")
    lkB = nc.dram_tensor("lkB", (RANK, HD), F32, kind="ExternalInput")  # pre-scaled
    lvA = nc.dram_tensor("lvA", (HD, RANK), F32, kind="ExternalInput")
    lvB = nc.dram_tensor("lvB", (RANK, HD), F32, kind="ExternalInput")  # pre-scaled
    ident = nc.dram_tensor("ident", (128, 128), F32, kind="ExternalInput")
    out = nc.dram_tensor("out", (T, D), BF16, kind="ExternalOutput")

    r = lambda ap: ap.bitcast(F32R)

    from contextlib import ExitStack
    with tile.TileContext(nc) as tc, ExitStack() as est:
        cp = est.enter_context(tc.tile_pool(name="consts", bufs=1))
        pp = est.enter_context(tc.tile_pool(name="persist", bufs=1))

        # pin the Act table that serves square+ln+exp, so the auto-insertion
        # pass doesn't thrash between natural_log and exp tables
        from concourse.hw_specs import get_activation_tables
        _tables = list(get_activation_tables(nc.m.arch).keys())
        _atl = mybir.InstLoadActFuncSet(
            name=nc.get_next_instruction_name(), ins=[], outs=[],
            act_func_set_id=_tables.index("natural_log_exp_and_others"))
        _atl.engine = mybir.EngineType.Activation
        nc.scalar.add_instruction(_atl)

        # ---- small consts ----
        onesb = cp.tile([128, 1], BF16)
        nc.vector.memset(onesb[:], 1.0)
        epsc = cp.tile([128, 1], F32)
        nc.vector.memset(epsc[:], EPS)
        zeroc = cp.tile([128, 1], F32)
        nc.vector.memset(zeroc[:], 0.0)
        mask_s = cp.tile([128, 128], F32)
        nc.sync.dma_start(mask_s[:], masktri[:])

        # ---- persistent activations ----
        # qT[h]: rope'd queries, [HD, T] bf16; aliased as oT (attention output)
        qT = [pp.tile([HD, T], BF16, tag=f"qT{h}", name=f"qT{h}") for h in range(NQH)]
        oT = qT
        kT = pp.tile([HD, R + T], BF16)           # cols 0:64 = adapted virtual keys
        vnat = pp.tile([128, NKB + 1, 128], BF16)  # block 0 = virtual values (rows 0:64)

        # ---- rope/norm consts (weighted cos/sin) ----
        cwq_s = cp.tile([HD, T], BF16)
        swq_s = cp.tile([HD, T], BF16)
        cwk_s = cp.tile([HD, T], BF16)
        swk_s = cp.tile([HD, T], BF16)

        # ---- weights in SBUF ----
        wqk_s = cp.tile([128, ND, NQH * HD + HD], BF16)   # q cols 0:512, k cols 512:640
        wv_s = cp.tile([128, ND, HD], BF16)
        wo_s = cp.tile([128, NQH, D], BF16)

        # ================= Phase 0: LoRA-adapt virtual KV (tiny) =================
        with tc.tile_pool(name="lora_ps", bufs=1, space="PSUM") as lps, \
             tc.tile_pool(name="lora_sb", bufs=1) as lsb:
            vkT_s = lsb.tile([HD, R], F32R)
            vvT_s = lsb.tile([HD, R], F32R)
            lkA_s = lsb.tile([HD, RANK], F32R)
            lkB_s = lsb.tile([RANK, HD], F32R)
            lvA_s = lsb.tile([HD, RANK], F32R)
            lvB_s = lsb.tile([RANK, HD], F32R)
            ident_s = lsb.tile([128, 128], F32R)
            nc.sync.dma_start(vkT_s[:], r(vkT[:]))
            nc.sync.dma_start(vvT_s[:], r(vvT[:]))
            nc.sync.dma_start(lkA_s[:], r(lkA[:]))
            nc.sync.dma_start(lkB_s[:], r(lkB[:]))
            nc.sync.dma_start(lvA_s[:], r(lvA[:]))
            nc.sync.dma_start(lvB_s[:], r(lvB[:]))
            nc.sync.dma_start(ident_s[:], r(ident[:]))
            # keys: kT[:, 0:64] = vkT + Bk^T Ak^T vkT  (Bk pre-scaled)
            t1 = lps.tile([RANK, R], F32, tag="l1")
            nc.tensor.matmul(t1[:], lkA_s[:], vkT_s[:], start=True, stop=True)
            t1s = lsb.tile([RANK, R], F32R)
            nc.scalar.copy(t1s[:], t1[:])
            t2 = lps.tile([HD, R], F32, tag="l2")
            nc.tensor.matmul(t2[:], lkB_s[:], t1s[:], start=True, stop=True)
            nc.vector.tensor_add(kT[:, 0:R], vkT_s[:].bitcast(F32), t2[:])
            # values
            u1 = lps.tile([RANK, R], F32, tag="l1")
            nc.tensor.matmul(u1[:], lvA_s[:], vvT_s[:], start=True, stop=True)
            u1s = lsb.tile([RANK, R], F32R)
            nc.scalar.copy(u1s[:], u1[:])
            u2 = lps.tile([HD, R], F32, tag="l2")
            nc.tensor.matmul(u2[:], lvB_s[:], u1s[:], start=True, stop=True)
            vvirt = lsb.tile([HD, R], F32R)
            with nc.allow_low_precision(reason="f32r same width as f32"):
                nc.vector.tensor_add(vvirt[:], vvT_s[:].bitcast(F32), u2[:])
            # transpose virtual values to natural layout -> vnat[0:64, 0, :]
            vtp = lps.tile([R, HD], F32R, tag="l3")
            nc.tensor.transpose(vtp[:], vvirt[:], ident_s[:])
            nc.gpsimd.tensor_copy(vnat[0:R, 0, :], vtp[:].bitcast(F32))

        # ---- weight / rope-const loads (after lora pool closes) ----
        nc.sync.dma_start(cwq_s[:], cwq[:])
        nc.sync.dma_start(swq_s[:], swq[:])
        nc.sync.dma_start(cwk_s[:], cwk[:])
        nc.sync.dma_start(swk_s[:], swk[:])
        # batched weight loads: one 3D-AP DMA per tensor (partition-major view)
        nc.sync.dma_start(wqk_s[:, :, 0:NQH * HD],
                          wq[:, :].rearrange("(n p) c -> p n c", p=128))
        nc.sync.dma_start(wqk_s[:, :, NQH * HD:],
                          wk[:, :].rearrange("(n p) c -> p n c", p=128))
        nc.sync.dma_start(wv_s[:],
                          wv[:, :].rearrange("(n p) c -> p n c", p=128))
        nc.sync.dma_start(wo_s[:],
                          wo[:, :].rearrange("(n p) c -> p n c", p=128))

        # ================= main chunk pipeline =================
        with tc.tile_pool(name="proj_ps", bufs=2, space="PSUM") as prps, \
             tc.tile_pool(name="mm_ps", bufs=3, space="PSUM") as mmps, \
             tc.tile_pool(name="den_ps", bufs=1, space="PSUM") as dnps, \
             tc.tile_pool(name="b2k_ps", bufs=2, space="PSUM") as b2ps, \
             tc.tile_pool(name="hs_sb", bufs=1) as hsb, \
             tc.tile_pool(name="nrm_sb", bufs=2) as nsb, \
             tc.tile_pool(name="pe_sb", bufs=6) as peb, \
             tc.tile_pool(name="at_sb", bufs=2) as asb, \
             tc.tile_pool(name="ob_sb", bufs=2) as obb:
            hs_tiles = {0: hsb.tile([128, ND, TC], BF16, tag="hs", name="hs0")}
            nc.sync.dma_start(hs_tiles[0][:],
                              hsT[:, 0:TC].rearrange("(n p) t -> p n t", p=128))
            for c in range(NTC):
                ts = slice(c * TC, (c + 1) * TC)
                hs_c = hs_tiles.pop(c)

                # ---- projections: 5 passes (q0..q3, k), each one accumulator ----
                for p in range(NQH + 1):
                    pacc = prps.tile([128, TC], F32, tag="pacc")
                    wslice = wqk_s[:, :, p * HD:(p + 1) * HD]
                    for d in range(ND):
                        nc.tensor.matmul(pacc[:], wslice[:, d, :], hs_c[:, d, :],
                                         start=(d == 0), stop=(d == ND - 1))
                    # ---- rms-norm + rope on this pass's PSUM ----
                    isq = p < NQH
                    cw = cwq_s if isq else cwk_s
                    sw = swq_s if isq else swk_s
                    dst = qT[p][:, ts] if isq else kT[:, R + c * TC: R + (c + 1) * TC]
                    sq = nsb.tile([HD, TC], BF16, tag="sq")
                    nc.gpsimd.tensor_mul(sq[:], pacc[:], pacc[:])
                    ssum = nsb.tile([HD, TC], F32, tag="ssum")
                    nc.gpsimd.partition_all_reduce(ssum[:], sq[:], channels=128,
                                                   reduce_op=RED.add)
                    lns = nsb.tile([HD, TC], F32, tag="lns")
                    nc.scalar.activation(lns[:], ssum[:], ACTF.Ln,
                                         scale=1.0 / HD, bias=epsc[:])
                    rinv = nsb.tile([HD, TC], F32, tag="rinv")
                    nc.scalar.activation(rinv[:], lns[:], ACTF.Exp, scale=-0.5,
                                         bias=zeroc[:])
                    xn = nsb.tile([HD, TC], BF16, tag="xn")
                    nc.vector.tensor_mul(xn[:], pacc[:], rinv[:])
                    t1 = nsb.tile([HD, TC], BF16, tag="t1")
                    nc.vector.tensor_mul(t1[:], xn[:], cw[:, ts])
                    t2 = nsb.tile([HD, TC], BF16, tag="t2")
                    nc.vector.tensor_mul(t2[0:64, :], xn[64:128, :], sw[0:64, ts])
                    nc.vector.tensor_mul(t2[64:128, :], xn[0:64, :], sw[64:128, ts])
                    nc.vector.tensor_add(dst, t1[:], t2[:])

                # ---- V in natural layout: stationary = hs t-slices ----
                vacc = b2ps.tile([128, 4, 128], F32, tag="b2k")
                for tt in range(4):
                    for d in range(ND):
                        nc.tensor.matmul(vacc[:, tt, :],
                                         hs_c[:, d, tt * 128:(tt + 1) * 128],
                                         wv_s[:, d, :],
                                         start=(d == 0), stop=(d == ND - 1))
                nc.gpsimd.tensor_copy(vnat[:, 1 + 4 * c: 5 + 4 * c, :], vacc[:])

                # prefetch next chunk's hidden states (after last hs_c reader issued)
                if c + 1 < NTC:
                    hs_tiles[c + 1] = hsb.tile([128, ND, TC], BF16, tag="hs",
                                                name=f"hs{c + 1}")
                    nc.sync.dma_start(
                        hs_tiles[c + 1][:],
                        hsT[:, (c + 1) * TC:(c + 2) * TC].rearrange(
                            "(n p) t -> p n t", p=128))

                # ---- attention for this chunk's queries ----
                for h in range(NQH):
                    den = dnps.tile([1, TC], F32, tag="den")
                    po = b2ps.tile([128, TC], F32, tag="b2k")
                    # virtual block (full width)
                    st_ = mmps.tile([128, TC], F32, tag="mm")
                    nc.tensor.matmul(st_[0:R, :], kT[:, 0:R], qT[h][:, ts],
                                     start=True, stop=True)
                    pe = peb.tile([128, TC], BF16, tag="pe")
                    nc.scalar.activation(pe[0:R, :], st_[0:R, :], ACTF.Exp,
                                         scale=SCALING, bias=zeroc[0:R, :])
                    nc.tensor.matmul(den[:], onesb[0:R, :], pe[0:R, :],
                                     start=True, stop=False)
                    nc.tensor.matmul(po[:], vnat[0:R, 0, :], pe[0:R, :],
                                     start=True, stop=False)
                    # full (past) key blocks
                    for bb in range(4 * c):
                        st_ = mmps.tile([128, TC], F32, tag="mm")
                        nc.tensor.matmul(st_[:], kT[:, R + bb * 128: R + (bb + 1) * 128],
                                         qT[h][:, ts], start=True, stop=True)
                        pe = peb.tile([128, TC], BF16, tag="pe")
                        nc.scalar.activation(pe[:], st_[:], ACTF.Exp,
                                             scale=SCALING, bias=zeroc[:])
                        nc.tensor.matmul(den[:], onesb[:], pe[:],
                                         start=False, stop=False)
                        nc.tensor.matmul(po[:], vnat[:, 1 + bb, :], pe[:],
                                         start=False, stop=False)
                    # diagonal blocks j=0..3: queries >= 128*j only
                    for j in range(4):
                        bb = 4 * c + j
                        W = TC - 128 * j
                        qs = slice(c * TC + 128 * j, (c + 1) * TC)
                        st_ = mmps.tile([128, TC], F32, tag="mm")
                        nc.tensor.matmul(st_[:, 0:W],
                                         kT[:, R + bb * 128: R + (bb + 1) * 128],
                                         qT[h][:, qs], start=True, stop=True)
                        # triangular mask on the first 128 cols of this region
                        nc.gpsimd.tensor_add(st_[:, 0:128], st_[:, 0:128], mask_s[:])
                        pe = peb.tile([128, TC], BF16, tag="pe")
                        nc.scalar.activation(pe[:, 0:W], st_[:, 0:W], ACTF.Exp,
                                             scale=SCALING, bias=zeroc[:])
                        last = (j == 3)
                        nc.tensor.matmul(den[:, 128 * j:], onesb[:], pe[:, 0:W],
                                         start=False, stop=last)
                        nc.tensor.matmul(po[:, 128 * j:], vnat[:, 1 + bb, :], pe[:, 0:W],
                                         start=False, stop=last)
                    # normalize: oT[h][:, ts] = po * (1/den[h]) broadcast
                    rc = asb.tile([1, TC], F32, tag="rc")
                    nc.vector.reciprocal(rc[:], den[:])
                    rb = asb.tile([128, TC], F32, tag="rb")
                    nc.gpsimd.partition_broadcast(rb[:], rc[:], channels=128)
                    nc.vector.tensor_mul(oT[h][:, ts], po[:], rb[:])

                # ---- output projection for this chunk's 4 t-tiles ----
                for tt in range(4 * c, 4 * c + 4):
                    ob = obb.tile([128, 8, TC], BF16, tag="ob")
                    for j2 in range(D // TC):
                        po2 = mmps.tile([128, TC], F32, tag="mm")
                        for h in range(NQH):
                            nc.tensor.matmul(po2[:], oT[h][:, tt * 128:(tt + 1) * 128],
                                             wo_s[:, h, j2 * TC:(j2 + 1) * TC],
                                             start=(h == 0), stop=(h == NQH - 1))
                        nc.gpsimd.tensor_copy(ob[:, j2, :], po2[:])
                    eng = nc.sync if tt % 2 == 0 else nc.scalar
                    eng.dma_start(out[tt * 128:(tt + 1) * 128, :], ob[:])

    nc.compile()
    return nc


_NC_CACHE = {}


def _get_nc():
    if "nc" not in _NC_CACHE:
        _NC_CACHE["nc"] = build_nc()
    return _NC_CACHE["nc"]


def _bf(x):
    return np.ascontiguousarray(x.astype(ml_dtypes.bfloat16))


def kernel(**inputs) -> np.ndarray:
    f = lambda k: np.asarray(inputs[k], np.float32)
    hs = f("hidden_states")[0]            # (T, D)
    vk = f("virtual_keys")[0]             # (HKV, R, HD)
    vv = f("virtual_values")[0]
    Wq, Wk, Wv, Wo = f("Wq"), f("Wk"), f("Wv"), f("Wo")
    qnw, knw = f("q_norm_w"), f("k_norm_w")
    lkA, lkB = f("lora_k_A"), f("lora_k_B")
    lvA, lvB = f("lora_v_A"), f("lora_v_B")
    sk = np.float32(np.asarray(inputs["scale_k"]))
    sv = np.float32(np.asarray(inputs["scale_v"]))
    cos, sin = f("cos"), f("sin")         # (T, HD)

    hsT = _bf(hs.T)
    # weighted cos/sin for fused (rms*w) + rope:
    #   cw[d,t] = w[d]*cos[t,d]
    #   sw[d,t] = -w[d+64]*sin[t,d]  (d<64);  w[d-64]*sin[t,d]  (d>=64)
    def cw_sw(w):
        cw = (cos.T * w[:, None]).astype(np.float32)
        sw = np.empty((HD, T), np.float32)
        sw[0:64] = -w[64:128, None] * sin.T[0:64]
        sw[64:128] = w[0:64, None] * sin.T[64:128]
        return _bf(cw), _bf(sw)
    cwqh, swqh = cw_sw(qnw)
    cwkh, swkh = cw_sw(knw)
    # constant [128,128] triangular mask: allowed k<=q, else -1e30
    idx = np.arange(128)
    masktri = np.where(idx[:, None] <= idx[None, :], 0.0, -1e30).astype(np.float32)
    ident = np.eye(128, dtype=np.float32)
    lkBs = np.ascontiguousarray(lkB * sk)
    lvBs = np.ascontiguousarray(lvB * sv)

    in_maps = []
    for m in range(8):
        in_maps.append({
            "hsT": hsT,
            "wq": _bf(Wq[:, 512 * m:512 * (m + 1)]),
            "wk": _bf(Wk[:, 128 * m:128 * (m + 1)]),
            "wv": _bf(Wv[:, 128 * m:128 * (m + 1)]),
            "wo": _bf(Wo[512 * m:512 * (m + 1), :]),
            "cwq": cwqh, "swq": swqh, "cwk": cwkh, "swk": swkh,
            "masktri": masktri,
            "vkT": np.ascontiguousarray(vk[m].T),
            "vvT": np.ascontiguousarray(vv[m].T),
            "lkA": lkA, "lkB": lkBs, "lvA": lvA, "lvB": lvBs,
            "ident": ident,
        })

    nc = _get_nc()
    res = run_bass_kernel_spmd(nc, in_maps, core_ids=list(range(8)))
    acc = res.results[0]["out"].astype(np.float32)
    for m in range(1, 8):
        acc = acc + res.results[m]["out"].astype(np.float32)
    return acc[None]  # (1, T, D)


# revision 12
# speedup vs baseline: 1.1218x; 1.1218x over previous
"""Trainium2 Bass kernel for KVAdapterInjector (Qwen3-style GQA attention with
LoRA-adapted virtual KV prefix).

Sharding: tensor-parallel over heads across 8 cores. Core m gets KV head m and
Q heads 4m..4m+3. Wq/Wk/Wv sharded on output dim, Wo on input dim; partial
outputs (bf16) summed on host.

v2 design notes (cost-model driven):
- All heavy matmuls in bf16 (1.0 cycles/row, immune to the fp32r ap<256
  penalty). PSUM accumulation stays fp32. Measured end-to-end bf16 error
  ~5.5e-3 (budget 2e-2). fp8 was measured at 2.7-5e-2 per stage: rejected.
- PE-row accounting puts the tensor engine at ~370us; all other engines are
  kept under ~150us: softmax denominators stay as ones-matmuls on PE, but
  rms-norm sum/broadcast use gpsimd partition_all_reduce/broadcast (Pool),
  rsqrt = exp(-0.5*ln(x)) on Act (single activation table: ln+exp+square),
  mask-adds and PSUM drains ride Pool, rope elementwise rides DVE in bf16
  (2x mode).
- Causal diagonal blocks are trimmed: block j of a 512-query chunk only
  computes queries >= 128*j, with a constant [128,128] triangular mask tile.
- Chunk-pipelined: proj(c) -> norm/rope(c) -> attention(c) -> outproj(c),
  with PSUM pools sized to exactly 8 banks so phases from adjacent chunks
  overlap across engines.
"""
import sys

sys.path.insert(0, "/opt/trn_rl_repo")

import numpy as np
import ml_dtypes

import concourse.bass as bass
import concourse.mybir as mybir
import concourse.tile as tile
from concourse import bacc
from concourse import bass_isa
from concourse.bass_utils import run_bass_kernel_spmd

F32 = mybir.dt.float32
F32R = mybir.dt.float32r
BF16 = mybir.dt.bfloat16
AX = mybir.AxisListType
ALU = mybir.AluOpType
ACTF = mybir.ActivationFunctionType
RED = bass_isa.ReduceOp

T = 2048
D = 4096
HD = 128
NQH = 4          # q heads per core
R = 64           # virtual tokens
RANK = 16
EPS = 1e-6
SCALING = HD ** -0.5
NTC = 4          # T chunks of 512
TC = 512
ND = D // 128    # 32 contraction tiles
NKB = T // 128   # 16 key blocks (real)


def build_nc():
    nc = bacc.Bacc(None, target_bir_lowering=False, debug=False)

    # ---- DRAM I/O (bf16 activations/weights prepared on host) ----
    hsT = nc.dram_tensor("hsT", (D, T), BF16, kind="ExternalInput")
    wp01 = nc.dram_tensor("wp01", (D, 256), BF16, kind="ExternalInput")
    wp23 = nc.dram_tensor("wp23", (D, 256), BF16, kind="ExternalInput")
    wpkv = nc.dram_tensor("wpkv", (D, 256), BF16, kind="ExternalInput")
    wo = nc.dram_tensor("wo", (NQH * HD, D), BF16, kind="ExternalInput")
    cwq = nc.dram_tensor("cwq", (HD, T), BF16, kind="ExternalInput")
    swq = nc.dram_tensor("swq", (HD, T), BF16, kind="ExternalInput")
    cwk = nc.dram_tensor("cwk", (HD, T), BF16, kind="ExternalInput")
    swk = nc.dram_tensor("swk", (HD, T), BF16, kind="ExternalInput")
    masktri = nc.dram_tensor("masktri", (128, 128), F32, kind="ExternalInput")
    vkT = nc.dram_tensor("vkT", (HD, R), F32, kind="ExternalInput")
    vvT = nc.dram_tensor("vvT", (HD, R), F32, kind="ExternalInput")
    lkA = nc.dram_tensor("lkA", (HD, RANK), F32, kind="ExternalInput")
    lkB = nc.dram_tensor("lkB", (RANK, HD), F32, kind="ExternalInput")  # pre-scaled
    lvA = nc.dram_tensor("lvA", (HD, RANK), F32, kind="ExternalInput")
    lvB = nc.dram_tensor("lvB", (RANK, HD), F32, kind="ExternalInput")  # pre-scaled
    ident = nc.dram_tensor("ident", (128, 128), F32, kind="ExternalInput")
    out = nc.dram_tensor("out", (T, D), BF16, kind="ExternalOutput")

    r = lambda ap: ap.bitcast(F32R)

    from contextlib import ExitStack
    with tile.TileContext(nc) as tc, ExitStack() as est:
        cp = est.enter_context(tc.tile_pool(name="consts", bufs=1))
        pp = est.enter_context(tc.tile_pool(name="persist", bufs=1))

        # pin the Act table that serves square+ln+exp, so the auto-insertion
        # pass doesn't thrash between natural_log and exp tables
        from concourse.hw_specs import get_activation_tables
        _tables = list(get_activation_tables(nc.m.arch).keys())
        _atl = mybir.InstLoadActFuncSet(
            name=nc.get_next_instruction_name(), ins=[], outs=[],
            act_func_set_id=_tables.index("natural_log_exp_and_others"))
        _atl.engine = mybir.EngineType.Activation
        nc.scalar.add_instruction(_atl)

        # ---- small consts ----
        onesb = cp.tile([128, 1], BF16)
        nc.vector.memset(onesb[:], 1.0)
        epsc = cp.tile([128, 1], F32)
        nc.vector.memset(epsc[:], EPS)
        zeroc = cp.tile([128, 1], F32)
        nc.vector.memset(zeroc[:], 0.0)
        mask_s = cp.tile([128, 128], F32)
        nc.scalar.dma_start(mask_s[:], masktri[:])

        # ---- persistent activations ----
        # qT[h]: rope'd queries, [HD, T] bf16; aliased as oT (attention output)
        qT = [pp.tile([HD, T], BF16, tag=f"qT{h}", name=f"qT{h}") for h in range(NQH)]
        oT = qT
        kT = pp.tile([HD, R + T], BF16)           # cols 0:64 = adapted virtual keys
        vnat = pp.tile([128, NKB + 1, 128], BF16)  # block 0 = virtual values (rows 0:64)

        # ---- rope/norm consts (weighted cos/sin) ----
        cwq_s = cp.tile([HD, T], BF16)
        swq_s = cp.tile([HD, T], BF16)
        cwk_s = cp.tile([HD, T], BF16)
        swk_s = cp.tile([HD, T], BF16)

        # ---- weights in SBUF ----
        wqkv_s = cp.tile([128, ND, 768], BF16)   # q 0:512, k 512:640, v 640:768
        wo_s = cp.tile([128, NQH, D], BF16)

        # ================= Phase 0: LoRA-adapt virtual KV (tiny) =================
        with tc.tile_pool(name="lora_ps", bufs=1, space="PSUM") as lps, \
             tc.tile_pool(name="lora_sb", bufs=1) as lsb:
            vkT_s = lsb.tile([HD, R], F32R)
            vvT_s = lsb.tile([HD, R], F32R)
            lkA_s = lsb.tile([HD, RANK], F32R)
            lkB_s = lsb.tile([RANK, HD], F32R)
            lvA_s = lsb.tile([HD, RANK], F32R)
            lvB_s = lsb.tile([RANK, HD], F32R)
            ident_s = lsb.tile([128, 128], F32R)
            nc.scalar.dma_start(vkT_s[:], r(vkT[:]))
            nc.scalar.dma_start(vvT_s[:], r(vvT[:]))
            nc.scalar.dma_start(lkA_s[:], r(lkA[:]))
            nc.scalar.dma_start(lkB_s[:], r(lkB[:]))
            nc.scalar.dma_start(lvA_s[:], r(lvA[:]))
            nc.scalar.dma_start(lvB_s[:], r(lvB[:]))
            nc.scalar.dma_start(ident_s[:], r(ident[:]))
            # keys: kT[:, 0:64] = vkT + Bk^T Ak^T vkT  (Bk pre-scaled)
            t1 = lps.tile([RANK, R], F32, tag="l1")
            nc.tensor.matmul(t1[:], lkA_s[:], vkT_s[:], start=True, stop=True)
            t1s = lsb.tile([RANK, R], F32R)
            nc.scalar.copy(t1s[:], t1[:])
            t2 = lps.tile([HD, R], F32, tag="l2")
            nc.tensor.matmul(t2[:], lkB_s[:], t1s[:], start=True, stop=True)
            nc.vector.tensor_add(kT[:, 0:R], vkT_s[:].bitcast(F32), t2[:])
            # values
            u1 = lps.tile([RANK, R], F32, tag="l1")
            nc.tensor.matmul(u1[:], lvA_s[:], vvT_s[:], start=True, stop=True)
            u1s = lsb.tile([RANK, R], F32R)
            nc.scalar.copy(u1s[:], u1[:])
            u2 = lps.tile([HD, R], F32, tag="l2")
            nc.tensor.matmul(u2[:], lvB_s[:], u1s[:], start=True, stop=True)
            vvirt = lsb.tile([HD, R], F32R)
            with nc.allow_low_precision(reason="f32r same width as f32"):
                nc.vector.tensor_add(vvirt[:], vvT_s[:].bitcast(F32), u2[:])
            # transpose virtual values to natural layout -> vnat[0:64, 0, :]
            vtp = lps.tile([R, HD], F32R, tag="l3")
            nc.tensor.transpose(vtp[:], vvirt[:], ident_s[:])
            nc.gpsimd.tensor_copy(vnat[0:R, 0, :], vtp[:].bitcast(F32))

        # ---- weight / rope-const loads, ordered for earliest PE start ----
        pm = lambda ap: ap.rearrange("(n p) c -> p n c", p=128)
        nc.sync.dma_start(wqkv_s[:, :, 0:256], pm(wp01[:, :]))

        # ================= main chunk pipeline =================
        with tc.tile_pool(name="proj_ps", bufs=2, space="PSUM") as prps, \
             tc.tile_pool(name="mm_ps", bufs=3, space="PSUM") as mmps, \
             tc.tile_pool(name="den_ps", bufs=1, space="PSUM") as dnps, \
             tc.tile_pool(name="b2k_ps", bufs=2, space="PSUM") as b2ps, \
             tc.tile_pool(name="hs_sb", bufs=1) as hsb, \
             tc.tile_pool(name="nrm_sb", bufs=2) as nsb, \
             tc.tile_pool(name="pe_sb", bufs=6) as peb, \
             tc.tile_pool(name="at_sb", bufs=2) as asb, \
             tc.tile_pool(name="ob_sb", bufs=2) as obb:
            def load_hs(tile_, c_):
                for i in range(4):
                    nc.sync.dma_start(
                        tile_[:, 8 * i:8 * (i + 1), :],
                        hsT[1024 * i:1024 * (i + 1),
                            c_ * TC:(c_ + 1) * TC].rearrange(
                                "(n p) t -> p n t", p=128))
            hs_tiles = {0: hsb.tile([128, ND, TC], BF16, tag="hs", name="hs0")}
            load_hs(hs_tiles[0], 0)
            nc.sync.dma_start(wqkv_s[:, :, 256:512], pm(wp23[:, :]))
            nc.sync.dma_start(cwq_s[:], cwq[:])
            nc.sync.dma_start(swq_s[:], swq[:])
            nc.sync.dma_start(wqkv_s[:, :, 512:768], pm(wpkv[:, :]))
            nc.sync.dma_start(cwk_s[:], cwk[:])
            nc.sync.dma_start(swk_s[:], swk[:])
            nc.sync.dma_start(wo_s[:], pm(wo[:, :]))
            for c in range(NTC):
                ts = slice(c * TC, (c + 1) * TC)
                hs_c = hs_tiles.pop(c)

                # ---- projections: 5 passes (q0..q3, k), each one accumulator ----
                for p in range(NQH + 1):
                    pacc = prps.tile([128, TC], F32, tag="pacc")
                    wslice = wqkv_s[:, :, p * HD:(p + 1) * HD]
                    for d in range(ND):
                        nc.tensor.matmul(pacc[:], wslice[:, d, :], hs_c[:, d, :],
                                         start=(d == 0), stop=(d == ND - 1))
                    # ---- rms-norm + rope on this pass's PSUM ----
                    isq = p < NQH
                    cw = cwq_s if isq else cwk_s
                    sw = swq_s if isq else swk_s
                    dst = qT[p][:, ts] if isq else kT[:, R + c * TC: R + (c + 1) * TC]
                    sq = nsb.tile([HD, TC], BF16, tag="sq")
                    nc.gpsimd.tensor_mul(sq[:], pacc[:], pacc[:])
                    ssum = nsb.tile([HD, TC], F32, tag="ssum")
                    nc.gpsimd.partition_all_reduce(ssum[:], sq[:], channels=128,
                                                   reduce_op=RED.add)
                    lns = nsb.tile([HD, TC], F32, tag="lns")
                    nc.scalar.activation(lns[:], ssum[:], ACTF.Ln,
                                         scale=1.0 / HD, bias=epsc[:])
                    rinv = nsb.tile([HD, TC], F32, tag="rinv")
                    nc.scalar.activation(rinv[:], lns[:], ACTF.Exp, scale=-0.5,
                                         bias=zeroc[:])
                    xn = nsb.tile([HD, TC], BF16, tag="xn")
                    nc.vector.tensor_mul(xn[:], pacc[:], rinv[:])
                    t1 = nsb.tile([HD, TC], BF16, tag="t1")
                    nc.vector.tensor_mul(t1[:], xn[:], cw[:, ts])
                    t2 = nsb.tile([HD, TC], BF16, tag="t2")
                    nc.vector.tensor_mul(t2[0:64, :], xn[64:128, :], sw[0:64, ts])
                    nc.vector.tensor_mul(t2[64:128, :], xn[0:64, :], sw[64:128, ts])
                    nc.vector.tensor_add(dst, t1[:], t2[:])

                # ---- V in natural layout: stationary = hs t-slices ----
                vacc = b2ps.tile([128, 4, 128], F32, tag="b2k")
                for tt in range(4):
                    for d in range(ND):
                        nc.tensor.matmul(vacc[:, tt, :],
                                         hs_c[:, d, tt * 128:(tt + 1) * 128],
                                         wqkv_s[:, d, 640:768],
                                         start=(d == 0), stop=(d == ND - 1))
                nc.gpsimd.tensor_copy(vnat[:, 1 + 4 * c: 5 + 4 * c, :], vacc[:])

                # prefetch next chunk's hidden states (after last hs_c reader issued)
                if c + 1 < NTC:
                    hs_tiles[c + 1] = hsb.tile([128, ND, TC], BF16, tag="hs",
                                                name=f"hs{c + 1}")
                    load_hs(hs_tiles[c + 1], c + 1)

                # ---- attention for this chunk's queries ----
                for h in range(NQH):
                    den = dnps.tile([1, TC], F32, tag="den")
                    po = b2ps.tile([128, TC], F32, tag="b2k")
                    # virtual block (full width)
                    st_ = mmps.tile([128, TC], F32, tag="mm")
                    nc.tensor.matmul(st_[0:R, :], kT[:, 0:R], qT[h][:, ts],
                                     start=True, stop=True)
                    pe = peb.tile([128, TC], BF16, tag="pe")
                    nc.scalar.activation(pe[0:R, :], st_[0:R, :], ACTF.Exp,
                                         scale=SCALING, bias=zeroc[0:R, :])
                    nc.tensor.matmul(den[:], onesb[0:R, :], pe[0:R, :],
                                     start=True, stop=False)
                    nc.tensor.matmul(po[:], vnat[0:R, 0, :], pe[0:R, :],
                                     start=True, stop=False)
                    # full (past) key blocks
                    for bb in range(4 * c):
                        st_ = mmps.tile([128, TC], F32, tag="mm")
                        nc.tensor.matmul(st_[:], kT[:, R + bb * 128: R + (bb + 1) * 128],
                                         qT[h][:, ts], start=True, stop=True)
                        pe = peb.tile([128, TC], BF16, tag="pe")
                        nc.scalar.activation(pe[:], st_[:], ACTF.Exp,
                                             scale=SCALING, bias=zeroc[:])
                        nc.tensor.matmul(den[:], onesb[:], pe[:],
                                         start=False, stop=False)
                        nc.tensor.matmul(po[:], vnat[:, 1 + bb, :], pe[:],
                                         start=False, stop=False)
                    # diagonal blocks j=0..3: queries >= 128*j only
                    for j in range(4):
                        bb = 4 * c + j
                        W = TC - 128 * j
                        qs = slice(c * TC + 128 * j, (c + 1) * TC)
                        st_ = mmps.tile([128, TC], F32, tag="mm")
                        nc.tensor.matmul(st_[:, 0:W],
                                         kT[:, R + bb * 128: R + (bb + 1) * 128],
                                         qT[h][:, qs], start=True, stop=True)
                        # triangular mask on the first 128 cols of this region
                        nc.gpsimd.tensor_add(st_[:, 0:128], st_[:, 0:128], mask_s[:])
                        pe = peb.tile([128, TC], BF16, tag="pe")
                        nc.scalar.activation(pe[:, 0:W], st_[:, 0:W], ACTF.Exp,
                                             scale=SCALING, bias=zeroc[:])
                        last = (j == 3)
                        nc.tensor.matmul(den[:, 128 * j:], onesb[:], pe[:, 0:W],
                                         start=False, stop=last)
                        nc.tensor.matmul(po[:, 128 * j:], vnat[:, 1 + bb, :], pe[:, 0:W],
                                         start=False, stop=last)
                    # normalize: oT[h][:, ts] = po * (1/den[h]) broadcast
                    rc = asb.tile([1, TC], F32, tag="rc")
                    nc.vector.reciprocal(rc[:], den[:])
                    rb = asb.tile([128, TC], F32, tag="rb")
                    nc.gpsimd.partition_broadcast(rb[:], rc[:], channels=128)
                    nc.vector.tensor_mul(oT[h][:, ts], po[:], rb[:])

                # ---- output projection for this chunk's 4 t-tiles ----
                for tt in range(4 * c, 4 * c + 4):
                    ob = obb.tile([128, 8, TC], BF16, tag="ob")
                    eng = nc.sync if tt % 2 == 0 else nc.scalar
                    for j2 in range(D // TC):
                        po2 = mmps.tile([128, TC], F32, tag="mm")
                        for h in range(NQH):
                            nc.tensor.matmul(po2[:], oT[h][:, tt * 128:(tt + 1) * 128],
                                             wo_s[:, h, j2 * TC:(j2 + 1) * TC],
                                             start=(h == 0), stop=(h == NQH - 1))
                        nc.gpsimd.tensor_copy(ob[:, j2, :], po2[:])
                        if j2 == 3:
                            eng.dma_start(out[tt * 128:(tt + 1) * 128, 0:2048],
                                          ob[:, 0:4, :])
                    eng.dma_start(out[tt * 128:(tt + 1) * 128, 2048:4096],
                                  ob[:, 4:8, :])

    nc.compile()
    return nc


_NC_CACHE = {}


def _get_nc():
    if "nc" not in _NC_CACHE:
        _NC_CACHE["nc"] = build_nc()
    return _NC_CACHE["nc"]


def _bf(x):
    return np.ascontiguousarray(x.astype(ml_dtypes.bfloat16))


def kernel(**inputs) -> np.ndarray:
    f = lambda k: np.asarray(inputs[k], np.float32)
    hs = f("hidden_states")[0]            # (T, D)
    vk = f("virtual_keys")[0]             # (HKV, R, HD)
    vv = f("virtual_values")[0]
    Wq, Wk, Wv, Wo = f("Wq"), f("Wk"), f("Wv"), f("Wo")
    qnw, knw = f("q_norm_w"), f("k_norm_w")
    lkA, lkB = f("lora_k_A"), f("lora_k_B")
    lvA, lvB = f("lora_v_A"), f("lora_v_B")
    sk = np.float32(np.asarray(inputs["scale_k"]))
    sv = np.float32(np.asarray(inputs["scale_v"]))
    cos, sin = f("cos"), f("sin")         # (T, HD)

    hsT = _bf(hs.T)
    # weighted cos/sin for fused (rms*w) + rope:
    #   cw[d,t] = w[d]*cos[t,d]
    #   sw[d,t] = -w[d+64]*sin[t,d]  (d<64);  w[d-64]*sin[t,d]  (d>=64)
    def cw_sw(w):
        cw = (cos.T * w[:, None]).astype(np.float32)
        sw = np.empty((HD, T), np.float32)
        sw[0:64] = -w[64:128, None] * sin.T[0:64]
        sw[64:128] = w[0:64, None] * sin.T[64:128]
        return _bf(cw), _bf(sw)
    cwqh, swqh = cw_sw(qnw)
    cwkh, swkh = cw_sw(knw)
    # constant [128,128] triangular mask: allowed k<=q, else -1e30
    idx = np.arange(128)
    masktri = np.where(idx[:, None] <= idx[None, :], 0.0, -1e30).astype(np.float32)
    ident = np.eye(128, dtype=np.float32)
    lkBs = np.ascontiguousarray(lkB * sk)
    lvBs = np.ascontiguousarray(lvB * sv)

    in_maps = []
    for m in range(8):
        in_maps.append({
            "hsT": hsT,
            "wp01": _bf(Wq[:, 512 * m:512 * m + 256]),
            "wp23": _bf(Wq[:, 512 * m + 256:512 * (m + 1)]),
            "wpkv": _bf(np.concatenate([Wk[:, 128 * m:128 * (m + 1)],
                                        Wv[:, 128 * m:128 * (m + 1)]], axis=1)),
            "wo": _bf(Wo[512 * m:512 * (m + 1), :]),
            "cwq": cwqh, "swq": swqh, "cwk": cwkh, "swk": swkh,
            "masktri": masktri,
            "vkT": np.ascontiguousarray(vk[m].T),
            "vvT": np.ascontiguousarray(vv[m].T),
            "lkA": lkA, "lkB": lkBs, "lvA": lvA, "lvB": lvBs,
            "ident": ident,
        })

    nc = _get_nc()
    res = run_bass_kernel_spmd(nc, in_maps, core_ids=list(range(8)))
    acc = res.results[0]["out"].astype(np.float32)
    for m in range(1, 8):
        acc = acc + res.results[m]["out"].astype(np.float32)
    return acc[None]  # (1, T, D)


# revision 14
# speedup vs baseline: 1.1590x; 1.0332x over previous
"""Trainium2 Bass kernel for KVAdapterInjector (Qwen3-style GQA attention with
LoRA-adapted virtual KV prefix).

Sharding: tensor-parallel over heads across 8 cores. Core m gets KV head m and
Q heads 4m..4m+3. Wq/Wk/Wv sharded on output dim, Wo on input dim; partial
outputs (bf16) summed on host.

v2 design notes (cost-model driven):
- All heavy matmuls in bf16 (1.0 cycles/row, immune to the fp32r ap<256
  penalty). PSUM accumulation stays fp32. Measured end-to-end bf16 error
  ~5.5e-3 (budget 2e-2). fp8 was measured at 2.7-5e-2 per stage: rejected.
- PE-row accounting puts the tensor engine at ~370us; all other engines are
  kept under ~150us: softmax denominators stay as ones-matmuls on PE, but
  rms-norm sum/broadcast use gpsimd partition_all_reduce/broadcast (Pool),
  rsqrt = exp(-0.5*ln(x)) on Act (single activation table: ln+exp+square),
  mask-adds and PSUM drains ride Pool, rope elementwise rides DVE in bf16
  (2x mode).
- Causal diagonal blocks are trimmed: block j of a 512-query chunk only
  computes queries >= 128*j, with a constant [128,128] triangular mask tile.
- Chunk-pipelined: proj(c) -> norm/rope(c) -> attention(c) -> outproj(c),
  with PSUM pools sized to exactly 8 banks so phases from adjacent chunks
  overlap across engines.
"""
import sys

sys.path.insert(0, "/opt/trn_rl_repo")

import numpy as np
import ml_dtypes

import concourse.bass as bass
import concourse.mybir as mybir
import concourse.tile as tile
from concourse import bacc
from concourse import bass_isa
from concourse.bass_utils import run_bass_kernel_spmd

F32 = mybir.dt.float32
F32R = mybir.dt.float32r
BF16 = mybir.dt.bfloat16
AX = mybir.AxisListType
ALU = mybir.AluOpType
ACTF = mybir.ActivationFunctionType
RED = bass_isa.ReduceOp

T = 2048
D = 4096
HD = 128
NQH = 4          # q heads per core
R = 64           # virtual tokens
RANK = 16
EPS = 1e-6
SCALING = HD ** -0.5
NTC = 4          # T chunks of 512
TC = 512
ND = D // 128    # 32 contraction tiles
NKB = T // 128   # 16 key blocks (real)


def build_nc():
    nc = bacc.Bacc(None, target_bir_lowering=False, debug=False)

    # ---- DRAM I/O (bf16 activations/weights prepared on host) ----
    hsT = nc.dram_tensor("hsT", (D, T), BF16, kind="ExternalInput")
    wp01 = nc.dram_tensor("wp01", (D, 256), BF16, kind="ExternalInput")
    wp23 = nc.dram_tensor("wp23", (D, 256), BF16, kind="ExternalInput")
    wpkv = nc.dram_tensor("wpkv", (D, 256), BF16, kind="ExternalInput")
    wo = nc.dram_tensor("wo", (NQH * HD, D), BF16, kind="ExternalInput")
    cwq = nc.dram_tensor("cwq", (HD, T), BF16, kind="ExternalInput")
    swq = nc.dram_tensor("swq", (HD, T), BF16, kind="ExternalInput")
    cwk = nc.dram_tensor("cwk", (HD, T), BF16, kind="ExternalInput")
    swk = nc.dram_tensor("swk", (HD, T), BF16, kind="ExternalInput")
    masktri = nc.dram_tensor("masktri", (128, 128), F32, kind="ExternalInput")
    vkT = nc.dram_tensor("vkT", (HD, R), F32, kind="ExternalInput")
    vvT = nc.dram_tensor("vvT", (HD, R), F32, kind="ExternalInput")
    lkA = nc.dram_tensor("lkA", (HD, RANK), F32, kind="ExternalInput")
    lkB = nc.dram_tensor("lkB", (RANK, HD), F32, kind="ExternalInput")  # pre-scaled
    lvA = nc.dram_tensor("lvA", (HD, RANK), F32, kind="ExternalInput")
    lvB = nc.dram_tensor("lvB", (RANK, HD), F32, kind="ExternalInput")  # pre-scaled
    ident = nc.dram_tensor("ident", (128, 128), F32, kind="ExternalInput")
    out = nc.dram_tensor("out", (T, D), BF16, kind="ExternalOutput")

    r = lambda ap: ap.bitcast(F32R)

    from contextlib import ExitStack
    with tile.TileContext(nc) as tc, ExitStack() as est:
        cp = est.enter_context(tc.tile_pool(name="consts", bufs=1))
        pp = est.enter_context(tc.tile_pool(name="persist", bufs=1))

        # pin the Act table that serves square+ln+exp, so the auto-insertion
        # pass doesn't thrash between natural_log and exp tables
        from concourse.hw_specs import get_activation_tables
        _tables = list(get_activation_tables(nc.m.arch).keys())
        _atl = mybir.InstLoadActFuncSet(
            name=nc.get_next_instruction_name(), ins=[], outs=[],
            act_func_set_id=_tables.index("natural_log_exp_and_others"))
        _atl.engine = mybir.EngineType.Activation
        nc.scalar.add_instruction(_atl)

        # ---- small consts ----
        onesb = cp.tile([128, 1], BF16)
        nc.vector.memset(onesb[:], 1.0)
        epsc = cp.tile([128, 1], F32)
        nc.vector.memset(epsc[:], EPS)
        zeroc = cp.tile([128, 1], F32)
        nc.vector.memset(zeroc[:], 0.0)
        mask_s = cp.tile([128, 128], F32)
        nc.scalar.dma_start(mask_s[:], masktri[:])

        # ---- persistent activations ----
        # qT[h]: rope'd queries, [HD, T] bf16; aliased as oT (attention output)
        qT = [pp.tile([HD, T], BF16, tag=f"qT{h}", name=f"qT{h}") for h in range(NQH)]
        oT = qT
        kT = pp.tile([HD, R + T], BF16)           # cols 0:64 = adapted virtual keys
        vnat = pp.tile([128, NKB + 1, 128], BF16)  # block 0 = virtual values (rows 0:64)

        # ---- rope/norm consts (weighted cos/sin) ----
        cwq_s = cp.tile([HD, T], BF16)
        swq_s = cp.tile([HD, T], BF16)
        cwk_s = cp.tile([HD, T], BF16)
        swk_s = cp.tile([HD, T], BF16)

        # ---- weights in SBUF ----
        wqkv_s = cp.tile([128, ND, 768], BF16)   # q 0:512, k 512:640, v 640:768
        wo_s = cp.tile([128, NQH, D], BF16)

        # ================= Phase 0: LoRA-adapt virtual KV (tiny) =================
        with tc.tile_pool(name="lora_ps", bufs=1, space="PSUM") as lps, \
             tc.tile_pool(name="lora_sb", bufs=1) as lsb:
            vkT_s = lsb.tile([HD, R], F32R)
            vvT_s = lsb.tile([HD, R], F32R)
            lkA_s = lsb.tile([HD, RANK], F32R)
            lkB_s = lsb.tile([RANK, HD], F32R)
            lvA_s = lsb.tile([HD, RANK], F32R)
            lvB_s = lsb.tile([RANK, HD], F32R)
            ident_s = lsb.tile([128, 128], F32R)
            nc.scalar.dma_start(vkT_s[:], r(vkT[:]))
            nc.scalar.dma_start(vvT_s[:], r(vvT[:]))
            nc.scalar.dma_start(lkA_s[:], r(lkA[:]))
            nc.scalar.dma_start(lkB_s[:], r(lkB[:]))
            nc.scalar.dma_start(lvA_s[:], r(lvA[:]))
            nc.scalar.dma_start(lvB_s[:], r(lvB[:]))
            nc.scalar.dma_start(ident_s[:], r(ident[:]))
            # keys: kT[:, 0:64] = vkT + Bk^T Ak^T vkT  (Bk pre-scaled)
            t1 = lps.tile([RANK, R], F32, tag="l1")
            nc.tensor.matmul(t1[:], lkA_s[:], vkT_s[:], start=True, stop=True)
            t1s = lsb.tile([RANK, R], F32R)
            nc.scalar.copy(t1s[:], t1[:])
            t2 = lps.tile([HD, R], F32, tag="l2")
            nc.tensor.matmul(t2[:], lkB_s[:], t1s[:], start=True, stop=True)
            nc.vector.tensor_add(kT[:, 0:R], vkT_s[:].bitcast(F32), t2[:])
            # values
            u1 = lps.tile([RANK, R], F32, tag="l1")
            nc.tensor.matmul(u1[:], lvA_s[:], vvT_s[:], start=True, stop=True)
            u1s = lsb.tile([RANK, R], F32R)
            nc.scalar.copy(u1s[:], u1[:])
            u2 = lps.tile([HD, R], F32, tag="l2")
            nc.tensor.matmul(u2[:], lvB_s[:], u1s[:], start=True, stop=True)
            vvirt = lsb.tile([HD, R], F32R)
            with nc.allow_low_precision(reason="f32r same width as f32"):
                nc.vector.tensor_add(vvirt[:], vvT_s[:].bitcast(F32), u2[:])
            # transpose virtual values to natural layout -> vnat[0:64, 0, :]
            vtp = lps.tile([R, HD], F32R, tag="l3")
            nc.tensor.transpose(vtp[:], vvirt[:], ident_s[:])
            nc.gpsimd.tensor_copy(vnat[0:R, 0, :], vtp[:].bitcast(F32))

        # ---- weight / rope-const loads, ordered for earliest PE start ----
        pm = lambda ap: ap.rearrange("(n p) c -> p n c", p=128)
        nc.sync.dma_start(wqkv_s[:, :, 0:256], pm(wp01[:, :]))

        # ================= main chunk pipeline =================
        with tc.tile_pool(name="proj_ps", bufs=2, space="PSUM") as prps, \
             tc.tile_pool(name="mm_ps", bufs=4, space="PSUM") as mmps, \
             tc.tile_pool(name="b2k_ps", bufs=2, space="PSUM") as b2ps, \
             tc.tile_pool(name="hs_sb", bufs=1) as hsb, \
             tc.tile_pool(name="nrm_sb", bufs=2) as nsb, \
             tc.tile_pool(name="pe_sb", bufs=6) as peb, \
             tc.tile_pool(name="at_sb", bufs=2) as asb, \
             tc.tile_pool(name="ob_sb", bufs=2) as obb:
            def new_hs(c_):
                return [hsb.tile([128, 8, TC], BF16, tag=f"hs{i}",
                                 name=f"hs{c_}_{i}") for i in range(4)]
            def load_hs(tiles_, c_):
                for i in range(4):
                    nc.sync.dma_start(
                        tiles_[i][:],
                        hsT[1024 * i:1024 * (i + 1),
                            c_ * TC:(c_ + 1) * TC].rearrange(
                                "(n p) t -> p n t", p=128))
            hs_tiles = {0: new_hs(0)}
            load_hs(hs_tiles[0], 0)
            nc.sync.dma_start(wqkv_s[:, :, 256:512], pm(wp23[:, :]))
            nc.sync.dma_start(cwq_s[:], cwq[:])
            nc.sync.dma_start(swq_s[:], swq[:])
            nc.sync.dma_start(wqkv_s[:, :, 512:768], pm(wpkv[:, :]))
            nc.sync.dma_start(cwk_s[:], cwk[:])
            nc.sync.dma_start(swk_s[:], swk[:])
            nc.sync.dma_start(wo_s[:], pm(wo[:, :]))
            for c in range(NTC):
                ts = slice(c * TC, (c + 1) * TC)
                hs_c = hs_tiles.pop(c)

                # ---- projections: 5 passes (q0..q3, k), each one accumulator ----
                for p in range(NQH + 1):
                    pacc = prps.tile([128, TC], F32, tag="pacc")
                    wslice = wqkv_s[:, :, p * HD:(p + 1) * HD]
                    for d in range(ND):
                        nc.tensor.matmul(pacc[:], wslice[:, d, :],
                                         hs_c[d // 8][:, d % 8, :],
                                         start=(d == 0), stop=(d == ND - 1))
                    # ---- rms-norm + rope on this pass's PSUM ----
                    isq = p < NQH
                    cw = cwq_s if isq else cwk_s
                    sw = swq_s if isq else swk_s
                    dst = qT[p][:, ts] if isq else kT[:, R + c * TC: R + (c + 1) * TC]
                    sq = nsb.tile([HD, TC], BF16, tag="sq")
                    nc.gpsimd.tensor_mul(sq[:], pacc[:], pacc[:])
                    ssum = nsb.tile([HD, TC], BF16, tag="ssum")
                    nc.gpsimd.partition_all_reduce(ssum[:], sq[:], channels=128,
                                                   reduce_op=RED.add)
                    lns = nsb.tile([HD, TC], F32, tag="lns")
                    nc.scalar.activation(lns[:], ssum[:], ACTF.Ln,
                                         scale=1.0 / HD, bias=epsc[:])
                    rinv = nsb.tile([HD, TC], BF16, tag="rinv")
                    nc.scalar.activation(rinv[:], lns[:], ACTF.Exp, scale=-0.5,
                                         bias=zeroc[:])
                    xn = nsb.tile([HD, TC], BF16, tag="xn")
                    nc.vector.tensor_mul(xn[:], pacc[:], rinv[:])
                    t1 = nsb.tile([HD, TC], BF16, tag="t1")
                    nc.vector.tensor_mul(t1[:], xn[:], cw[:, ts])
                    t2 = nsb.tile([HD, TC], BF16, tag="t2")
                    nc.vector.tensor_mul(t2[0:64, :], xn[64:128, :], sw[0:64, ts])
                    nc.vector.tensor_mul(t2[64:128, :], xn[0:64, :], sw[64:128, ts])
                    nc.vector.tensor_add(dst, t1[:], t2[:])

                # ---- V in natural layout: stationary = hs t-slices ----
                vacc = b2ps.tile([128, 4, 128], F32, tag="b2k")
                for tt in range(4):
                    for d in range(ND):
                        nc.tensor.matmul(vacc[:, tt, :],
                                         hs_c[d // 8][:, d % 8,
                                                      tt * 128:(tt + 1) * 128],
                                         wqkv_s[:, d, 640:768],
                                         start=(d == 0), stop=(d == ND - 1))
                nc.gpsimd.tensor_copy(vnat[:, 1 + 4 * c: 5 + 4 * c, :], vacc[:])

                # prefetch next chunk's hidden states (after last hs_c reader issued)
                if c + 1 < NTC:
                    hs_tiles[c + 1] = new_hs(c + 1)
                    load_hs(hs_tiles[c + 1], c + 1)

                # ---- attention for this chunk's queries ----
                for h in range(NQH):
                    den = asb.tile([1, TC], F32, tag="den", bufs=2)
                    po = b2ps.tile([128, TC], F32, tag="b2k")

                    def blocksum(pe_ap, first, wslc):
                        # denominator accumulation off-PE: partition-reduce
                        # on Pool, then row-add into den on DVE
                        ps_ = peb.tile([128, TC], F32, tag="ps", bufs=3)
                        rows = pe_ap.shape[0]
                        W_ = pe_ap.shape[-1]
                        nc.gpsimd.partition_all_reduce(
                            ps_[0:rows, 0:W_], pe_ap, channels=rows,
                            reduce_op=RED.add)
                        if first:
                            nc.vector.tensor_copy(den[:], ps_[0:1, 0:W_])
                        else:
                            nc.vector.tensor_add(den[:, wslc], den[:, wslc],
                                                 ps_[0:1, 0:W_])

                    # virtual block (full width)
                    st_ = mmps.tile([128, TC], F32, tag="mm")
                    nc.tensor.matmul(st_[0:R, :], kT[:, 0:R], qT[h][:, ts],
                                     start=True, stop=True)
                    pe = peb.tile([128, TC], BF16, tag="pe")
                    nc.scalar.activation(pe[0:R, :], st_[0:R, :], ACTF.Exp,
                                         scale=SCALING, bias=zeroc[0:R, :])
                    blocksum(pe[0:R, :], True, slice(0, TC))
                    nc.tensor.matmul(po[:], vnat[0:R, 0, :], pe[0:R, :],
                                     start=True, stop=False)
                    # full (past) key blocks
                    for bb in range(4 * c):
                        st_ = mmps.tile([128, TC], F32, tag="mm")
                        nc.tensor.matmul(st_[:], kT[:, R + bb * 128: R + (bb + 1) * 128],
                                         qT[h][:, ts], start=True, stop=True)
                        pe = peb.tile([128, TC], BF16, tag="pe")
                        nc.scalar.activation(pe[:], st_[:], ACTF.Exp,
                                             scale=SCALING, bias=zeroc[:])
                        blocksum(pe[:], False, slice(0, TC))
                        nc.tensor.matmul(po[:], vnat[:, 1 + bb, :], pe[:],
                                         start=False, stop=False)
                    # diagonal blocks j=0..3: queries >= 128*j only
                    for j in range(4):
                        bb = 4 * c + j
                        W = TC - 128 * j
                        qs = slice(c * TC + 128 * j, (c + 1) * TC)
                        st_ = mmps.tile([128, TC], F32, tag="mm")
                        nc.tensor.matmul(st_[:, 0:W],
                                         kT[:, R + bb * 128: R + (bb + 1) * 128],
                                         qT[h][:, qs], start=True, stop=True)
                        # triangular mask on the first 128 cols of this region
                        nc.gpsimd.tensor_add(st_[:, 0:128], st_[:, 0:128], mask_s[:])
                        pe = peb.tile([128, TC], BF16, tag="pe")
                        nc.scalar.activation(pe[:, 0:W], st_[:, 0:W], ACTF.Exp,
                                             scale=SCALING, bias=zeroc[:])
                        last = (j == 3)
                        blocksum(pe[:, 0:W], False, slice(128 * j, TC))
                        nc.tensor.matmul(po[:, 128 * j:], vnat[:, 1 + bb, :], pe[:, 0:W],
                                         start=False, stop=last)
                    # normalize: oT[h][:, ts] = po * (1/den[h]) broadcast
                    rc = asb.tile([1, TC], F32, tag="rc")
                    nc.vector.reciprocal(rc[:], den[:])
                    rb = asb.tile([128, TC], F32, tag="rb")
                    nc.gpsimd.partition_broadcast(rb[:], rc[:], channels=128)
                    nc.vector.tensor_mul(oT[h][:, ts], po[:], rb[:])

                # ---- output projection for this chunk's 4 t-tiles ----
                for tt in range(4 * c, 4 * c + 4):
                    eng = nc.sync if tt % 2 == 0 else nc.scalar
                    for half in range(2):
                        ob = obb.tile([128, 4, TC], BF16, tag="ob")
                        for jj in range(4):
                            j2 = 4 * half + jj
                            po2 = mmps.tile([128, TC], F32, tag="mm")
                            for h in range(NQH):
                                nc.tensor.matmul(
                                    po2[:], oT[h][:, tt * 128:(tt + 1) * 128],
                                    wo_s[:, h, j2 * TC:(j2 + 1) * TC],
                                    start=(h == 0), stop=(h == NQH - 1))
                            nc.gpsimd.tensor_copy(ob[:, jj, :], po2[:])
                        eng.dma_start(
                            out[tt * 128:(tt + 1) * 128,
                                half * 2048:(half + 1) * 2048], ob[:])

    nc.compile()
    return nc


_NC_CACHE = {}


def _get_nc():
    if "nc" not in _NC_CACHE:
        _NC_CACHE["nc"] = build_nc()
    return _NC_CACHE["nc"]


def _bf(x):
    return np.ascontiguousarray(x.astype(ml_dtypes.bfloat16))


def kernel(**inputs) -> np.ndarray:
    f = lambda k: np.asarray(inputs[k], np.float32)
    hs = f("hidden_states")[0]            # (T, D)
    vk = f("virtual_keys")[0]             # (HKV, R, HD)
    vv = f("virtual_values")[0]
    Wq, Wk, Wv, Wo = f("Wq"), f("Wk"), f("Wv"), f("Wo")
    qnw, knw = f("q_norm_w"), f("k_norm_w")
    lkA, lkB = f("lora_k_A"), f("lora_k_B")
    lvA, lvB = f("lora_v_A"), f("lora_v_B")
    sk = np.float32(np.asarray(inputs["scale_k"]))
    sv = np.float32(np.asarray(inputs["scale_v"]))
    cos, sin = f("cos"), f("sin")         # (T, HD)

    hsT = _bf(hs.T)
    # weighted cos/sin for fused (rms*w) + rope:
    #   cw[d,t] = w[d]*cos[t,d]
    #   sw[d,t] = -w[d+64]*sin[t,d]  (d<64);  w[d-64]*sin[t,d]  (d>=64)
    def cw_sw(w):
        cw = (cos.T * w[:, None]).astype(np.float32)
        sw = np.empty((HD, T), np.float32)
        sw[0:64] = -w[64:128, None] * sin.T[0:64]
        sw[64:128] = w[0:64, None] * sin.T[64:128]
        return _bf(cw), _bf(sw)
    cwqh, swqh = cw_sw(qnw)
    cwkh, swkh = cw_sw(knw)
    # constant [128,128] triangular mask: allowed k<=q, else -1e30
    idx = np.arange(128)
    masktri = np.where(idx[:, None] <= idx[None, :], 0.0, -1e30).astype(np.float32)
    ident = np.eye(128, dtype=np.float32)
    lkBs = np.ascontiguousarray(lkB * sk)
    lvBs = np.ascontiguousarray(lvB * sv)

    in_maps = []
    for m in range(8):
        in_maps.append({
            "hsT": hsT,
            "wp01": _bf(Wq[:, 512 * m:512 * m + 256]),
            "wp23": _bf(Wq[:, 512 * m + 256:512 * (m + 1)]),
            "wpkv": _bf(np.concatenate([Wk[:, 128 * m:128 * (m + 1)],
                                        Wv[:, 128 * m:128 * (m + 1)]], axis=1)),
            "wo": _bf(Wo[512 * m:512 * (m + 1), :]),
            "cwq": cwqh, "swq": swqh, "cwk": cwkh, "swk": swkh,
            "masktri": masktri,
            "vkT": np.ascontiguousarray(vk[m].T),
            "vvT": np.ascontiguousarray(vv[m].T),
            "lkA": lkA, "lkB": lkBs, "lvA": lvA, "lvB": lvBs,
            "ident": ident,
        })

    nc = _get_nc()
    res = run_bass_kernel_spmd(nc, in_maps, core_ids=list(range(8)))
    acc = res.results[0]["out"].astype(np.float32)
    for m in range(1, 8):
        acc = acc + res.results[m]["out"].astype(np.float32)
    return acc[None]  # (1, T, D)


# revision 16
# speedup vs baseline: 1.1800x; 1.0181x over previous
"""Trainium2 Bass kernel for KVAdapterInjector (Qwen3-style GQA attention with
LoRA-adapted virtual KV prefix).

Sharding: tensor-parallel over heads across 8 cores. Core m gets KV head m and
Q heads 4m..4m+3. Wq/Wk/Wv sharded on output dim, Wo on input dim; partial
outputs (bf16) summed on host.

v2 design notes (cost-model driven):
- All heavy matmuls in bf16 (1.0 cycles/row, immune to the fp32r ap<256
  penalty). PSUM accumulation stays fp32. Measured end-to-end bf16 error
  ~5.5e-3 (budget 2e-2). fp8 was measured at 2.7-5e-2 per stage: rejected.
- PE-row accounting puts the tensor engine at ~370us; all other engines are
  kept under ~150us: softmax denominators stay as ones-matmuls on PE, but
  rms-norm sum/broadcast use gpsimd partition_all_reduce/broadcast (Pool),
  rsqrt = exp(-0.5*ln(x)) on Act (single activation table: ln+exp+square),
  mask-adds and PSUM drains ride Pool, rope elementwise rides DVE in bf16
  (2x mode).
- Causal diagonal blocks are trimmed: block j of a 512-query chunk only
  computes queries >= 128*j, with a constant [128,128] triangular mask tile.
- Chunk-pipelined: proj(c) -> norm/rope(c) -> attention(c) -> outproj(c),
  with PSUM pools sized to exactly 8 banks so phases from adjacent chunks
  overlap across engines.
"""
import sys

sys.path.insert(0, "/opt/trn_rl_repo")

import numpy as np
import ml_dtypes

import concourse.bass as bass
import concourse.mybir as mybir
import concourse.tile as tile
from concourse import bacc
from concourse import bass_isa
from concourse.bass_utils import run_bass_kernel_spmd

F32 = mybir.dt.float32
F32R = mybir.dt.float32r
BF16 = mybir.dt.bfloat16
AX = mybir.AxisListType
ALU = mybir.AluOpType
ACTF = mybir.ActivationFunctionType
RED = bass_isa.ReduceOp

T = 2048
D = 4096
HD = 128
NQH = 4          # q heads per core
R = 64           # virtual tokens
RANK = 16
EPS = 1e-6
SCALING = HD ** -0.5
NTC = 4          # T chunks of 512
TC = 512
ND = D // 128    # 32 contraction tiles
NKB = T // 128   # 16 key blocks (real)


def build_nc():
    nc = bacc.Bacc(None, target_bir_lowering=False, debug=False)

    # ---- DRAM I/O (bf16 activations/weights prepared on host) ----
    wpp = nc.dram_tensor("wpp", (6, 128, ND * 128), BF16, kind="ExternalInput")
    hsp = nc.dram_tensor("hsp", (NTC * 4, 128, 8 * TC), BF16, kind="ExternalInput")
    wo = nc.dram_tensor("wo", (NQH * HD, D), BF16, kind="ExternalInput")
    cwq = nc.dram_tensor("cwq", (HD, T), BF16, kind="ExternalInput")
    swq = nc.dram_tensor("swq", (HD, T), BF16, kind="ExternalInput")
    cwk = nc.dram_tensor("cwk", (HD, T), BF16, kind="ExternalInput")
    swk = nc.dram_tensor("swk", (HD, T), BF16, kind="ExternalInput")
    masktri = nc.dram_tensor("masktri", (128, 128), F32, kind="ExternalInput")
    vkT = nc.dram_tensor("vkT", (HD, R), F32, kind="ExternalInput")
    vvT = nc.dram_tensor("vvT", (HD, R), F32, kind="ExternalInput")
    lkA = nc.dram_tensor("lkA", (HD, RANK), F32, kind="ExternalInput")
    lkB = nc.dram_tensor("lkB", (RANK, HD), F32, kind="ExternalInput")  # pre-scaled
    lvA = nc.dram_tensor("lvA", (HD, RANK), F32, kind="ExternalInput")
    lvB = nc.dram_tensor("lvB", (RANK, HD), F32, kind="ExternalInput")  # pre-scaled
    ident = nc.dram_tensor("ident", (128, 128), F32, kind="ExternalInput")
    out = nc.dram_tensor("out", (T, D), BF16, kind="ExternalOutput")

    r = lambda ap: ap.bitcast(F32R)

    from contextlib import ExitStack
    with tile.TileContext(nc) as tc, ExitStack() as est:
        cp = est.enter_context(tc.tile_pool(name="consts", bufs=1))
        pp = est.enter_context(tc.tile_pool(name="persist", bufs=1))

        # pin the Act table that serves square+ln+exp, so the auto-insertion
        # pass doesn't thrash between natural_log and exp tables
        from concourse.hw_specs import get_activation_tables
        _tables = list(get_activation_tables(nc.m.arch).keys())
        _atl = mybir.InstLoadActFuncSet(
            name=nc.get_next_instruction_name(), ins=[], outs=[],
            act_func_set_id=_tables.index("natural_log_exp_and_others"))
        _atl.engine = mybir.EngineType.Activation
        nc.scalar.add_instruction(_atl)

        # ---- small consts ----
        onesb = cp.tile([128, 1], BF16)
        nc.vector.memset(onesb[:], 1.0)
        epsc = cp.tile([128, 1], F32)
        nc.vector.memset(epsc[:], EPS)
        zeroc = cp.tile([128, 1], F32)
        nc.vector.memset(zeroc[:], 0.0)
        mask_s = cp.tile([128, 128], F32)
        nc.scalar.dma_start(mask_s[:], masktri[:])

        # ---- persistent activations ----
        # qT[h]: rope'd queries, [HD, T] bf16; aliased as oT (attention output)
        qT = [pp.tile([HD, T], BF16, tag=f"qT{h}", name=f"qT{h}") for h in range(NQH)]
        oT = qT
        kT = pp.tile([HD, R + T], BF16)           # cols 0:64 = adapted virtual keys
        vnat = pp.tile([128, NKB + 1, 128], BF16)  # block 0 = virtual values (rows 0:64)

        # ---- rope/norm consts (weighted cos/sin) ----
        cwq_s = cp.tile([HD, T], BF16)
        swq_s = cp.tile([HD, T], BF16)
        cwk_s = cp.tile([HD, T], BF16)
        swk_s = cp.tile([HD, T], BF16)

        # ---- weights in SBUF ----
        wqkv_s = cp.tile([128, 6, ND, 128], BF16)  # passes q0..q3, k, v
        wo_s = cp.tile([128, NQH, D], BF16)

        # ================= Phase 0: LoRA-adapt virtual KV (tiny) =================
        lsb = cp
        with tc.tile_pool(name="lora_ps", bufs=1, space="PSUM") as lps:
            vkT_s = lsb.tile([HD, R], F32R)
            vvT_s = lsb.tile([HD, R], F32R)
            lkA_s = lsb.tile([HD, RANK], F32R)
            lkB_s = lsb.tile([RANK, HD], F32R)
            lvA_s = lsb.tile([HD, RANK], F32R)
            lvB_s = lsb.tile([RANK, HD], F32R)
            ident_s = lsb.tile([128, 128], F32R)
            nc.scalar.dma_start(vkT_s[:], r(vkT[:]))
            nc.scalar.dma_start(vvT_s[:], r(vvT[:]))
            nc.scalar.dma_start(lkA_s[:], r(lkA[:]))
            nc.scalar.dma_start(lkB_s[:], r(lkB[:]))
            nc.scalar.dma_start(lvA_s[:], r(lvA[:]))
            nc.scalar.dma_start(lvB_s[:], r(lvB[:]))
            nc.scalar.dma_start(ident_s[:], r(ident[:]))
            # keys: kT[:, 0:64] = vkT + Bk^T Ak^T vkT  (Bk pre-scaled)
            t1 = lps.tile([RANK, R], F32, tag="l1")
            nc.tensor.matmul(t1[:], lkA_s[:], vkT_s[:], start=True, stop=True)
            t1s = lsb.tile([RANK, R], F32R)
            nc.scalar.copy(t1s[:], t1[:])
            t2 = lps.tile([HD, R], F32, tag="l2")
            nc.tensor.matmul(t2[:], lkB_s[:], t1s[:], start=True, stop=True)
            nc.vector.tensor_add(kT[:, 0:R], vkT_s[:].bitcast(F32), t2[:])
            # values
            u1 = lps.tile([RANK, R], F32, tag="l1")
            nc.tensor.matmul(u1[:], lvA_s[:], vvT_s[:], start=True, stop=True)
            u1s = lsb.tile([RANK, R], F32R)
            nc.scalar.copy(u1s[:], u1[:])
            u2 = lps.tile([HD, R], F32, tag="l2")
            nc.tensor.matmul(u2[:], lvB_s[:], u1s[:], start=True, stop=True)
            vvirt = lsb.tile([HD, R], F32R)
            with nc.allow_low_precision(reason="f32r same width as f32"):
                nc.vector.tensor_add(vvirt[:], vvT_s[:].bitcast(F32), u2[:])
            # transpose virtual values to natural layout -> vnat[0:64, 0, :]
            vtp = lps.tile([R, HD], F32R, tag="l3")
            nc.tensor.transpose(vtp[:], vvirt[:], ident_s[:])
            nc.gpsimd.tensor_copy(vnat[0:R, 0, :], vtp[:].bitcast(F32))

        # ---- weight / rope-const loads, ordered for earliest PE start ----
        pm = lambda ap: ap.rearrange("(n p) c -> p n c", p=128)
        nc.sync.dma_start(wqkv_s[:, 0, :, :], wpp[0])

        # ================= main chunk pipeline =================
        with tc.tile_pool(name="proj_ps", bufs=2, space="PSUM") as prps, \
             tc.tile_pool(name="mm_ps", bufs=4, space="PSUM") as mmps, \
             tc.tile_pool(name="b2k_ps", bufs=2, space="PSUM") as b2ps, \
             tc.tile_pool(name="hs_sb", bufs=1) as hsb, \
             tc.tile_pool(name="nrm_sb", bufs=2) as nsb, \
             tc.tile_pool(name="pe_sb", bufs=7) as peb, \
             tc.tile_pool(name="at_sb", bufs=2) as asb, \
             tc.tile_pool(name="ob_sb", bufs=2) as obb:
            def new_hs(c_):
                return [hsb.tile([128, 8, TC], BF16, tag=f"hs{i}",
                                 name=f"hs{c_}_{i}") for i in range(4)]
            def load_hs(tiles_, c_, engs=None):
                for i in range(4):
                    eng = nc.sync if engs is None else engs[i]
                    eng.dma_start(tiles_[i][:], hsp[4 * c_ + i])
            hs_tiles = {0: new_hs(0)}
            load_hs(hs_tiles[0], 0, engs=[nc.sync, nc.scalar, nc.sync, nc.scalar])
            nc.sync.dma_start(wqkv_s[:, 1, :, :], wpp[1])
            nc.sync.dma_start(wqkv_s[:, 2, :, :], wpp[2])
            nc.sync.dma_start(cwq_s[:], cwq[:])
            nc.sync.dma_start(swq_s[:], swq[:])
            nc.sync.dma_start(wqkv_s[:, 3, :, :], wpp[3])
            nc.sync.dma_start(wqkv_s[:, 4, :, :], wpp[4])
            nc.sync.dma_start(wqkv_s[:, 5, :, :], wpp[5])
            nc.sync.dma_start(cwk_s[:], cwk[:])
            nc.sync.dma_start(swk_s[:], swk[:])
            nc.sync.dma_start(wo_s[:], pm(wo[:, :]))
            for c in range(NTC):
                ts = slice(c * TC, (c + 1) * TC)
                hs_c = hs_tiles.pop(c)

                # ---- projections: 5 passes (q0..q3, k), each one accumulator ----
                for p in range(NQH + 1):
                    pacc = prps.tile([128, TC], F32, tag="pacc")
                    wslice = wqkv_s[:, p, :, :]
                    for d in range(ND):
                        nc.tensor.matmul(pacc[:], wslice[:, d, :],
                                         hs_c[d // 8][:, d % 8, :],
                                         start=(d == 0), stop=(d == ND - 1))
                    # ---- rms-norm + rope on this pass's PSUM ----
                    isq = p < NQH
                    cw = cwq_s if isq else cwk_s
                    sw = swq_s if isq else swk_s
                    dst = qT[p][:, ts] if isq else kT[:, R + c * TC: R + (c + 1) * TC]
                    sq = nsb.tile([HD, TC], BF16, tag="sq")
                    nc.gpsimd.tensor_mul(sq[:], pacc[:], pacc[:])
                    ssum = nsb.tile([HD, TC], BF16, tag="ssum")
                    nc.gpsimd.partition_all_reduce(ssum[:], sq[:], channels=128,
                                                   reduce_op=RED.add)
                    lns = nsb.tile([HD, TC], F32, tag="lns")
                    nc.scalar.activation(lns[:], ssum[:], ACTF.Ln,
                                         scale=1.0 / HD, bias=epsc[:])
                    rinv = nsb.tile([HD, TC], BF16, tag="rinv")
                    nc.scalar.activation(rinv[:], lns[:], ACTF.Exp, scale=-0.5,
                                         bias=zeroc[:])
                    xn = nsb.tile([HD, TC], BF16, tag="xn")
                    nc.vector.tensor_mul(xn[:], pacc[:], rinv[:])
                    t1 = nsb.tile([HD, TC], BF16, tag="t1")
                    nc.vector.tensor_mul(t1[:], xn[:], cw[:, ts])
                    t2 = nsb.tile([HD, TC], BF16, tag="t2")
                    nc.vector.tensor_mul(t2[0:64, :], xn[64:128, :], sw[0:64, ts])
                    nc.vector.tensor_mul(t2[64:128, :], xn[0:64, :], sw[64:128, ts])
                    nc.vector.tensor_add(dst, t1[:], t2[:])

                # ---- V in natural layout: stationary = hs t-slices ----
                vacc = b2ps.tile([128, 4, 128], F32, tag="b2k")
                for tt in range(4):
                    for d in range(ND):
                        nc.tensor.matmul(vacc[:, tt, :],
                                         hs_c[d // 8][:, d % 8,
                                                      tt * 128:(tt + 1) * 128],
                                         wqkv_s[:, 5, d, :],
                                         start=(d == 0), stop=(d == ND - 1))
                nc.gpsimd.tensor_copy(vnat[:, 1 + 4 * c: 5 + 4 * c, :], vacc[:])

                # prefetch next chunk's hidden states (after last hs_c reader issued)
                if c + 1 < NTC:
                    hs_tiles[c + 1] = new_hs(c + 1)
                    load_hs(hs_tiles[c + 1], c + 1)

                # ---- attention for this chunk's queries ----
                for h in range(NQH):
                    # two parity accumulators halve the serial add chain
                    denp = [asb.tile([1, TC], F32, tag="denE", bufs=2, name="denE"),
                            asb.tile([1, TC], F32, tag="denO", bufs=2, name="denO")]
                    blk_i = [0]
                    po = b2ps.tile([128, TC], F32, tag="b2k")

                    def blocksum(pe_ap, wslc):
                        # denominator accumulation off-PE: partition-reduce
                        # on Pool, then row-add into den parity acc on DVE
                        i = blk_i[0]; blk_i[0] += 1
                        den_ = denp[i % 2]
                        ps_ = peb.tile([128, TC], BF16, tag="ps", bufs=4)
                        rows = pe_ap.shape[0]
                        W_ = pe_ap.shape[-1]
                        nc.gpsimd.partition_all_reduce(
                            ps_[0:rows, 0:W_], pe_ap, channels=rows,
                            reduce_op=RED.add)
                        if i < 2:
                            nc.vector.tensor_copy(den_[:], ps_[0:1, 0:W_])
                        else:
                            nc.vector.tensor_add(den_[:, wslc], den_[:, wslc],
                                                 ps_[0:1, 0:W_])

                    # virtual block (full width)
                    st_ = mmps.tile([128, TC], F32, tag="mm")
                    nc.tensor.matmul(st_[0:R, :], kT[:, 0:R], qT[h][:, ts],
                                     start=True, stop=True)
                    pe = peb.tile([128, TC], BF16, tag="pe")
                    nc.scalar.activation(pe[0:R, :], st_[0:R, :], ACTF.Exp,
                                         scale=SCALING, bias=zeroc[0:R, :])
                    blocksum(pe[0:R, :], slice(0, TC))
                    nc.tensor.matmul(po[:], vnat[0:R, 0, :], pe[0:R, :],
                                     start=True, stop=False)
                    # full (past) key blocks
                    for bb in range(4 * c):
                        st_ = mmps.tile([128, TC], F32, tag="mm")
                        nc.tensor.matmul(st_[:], kT[:, R + bb * 128: R + (bb + 1) * 128],
                                         qT[h][:, ts], start=True, stop=True)
                        pe = peb.tile([128, TC], BF16, tag="pe")
                        nc.scalar.activation(pe[:], st_[:], ACTF.Exp,
                                             scale=SCALING, bias=zeroc[:])
                        blocksum(pe[:], slice(0, TC))
                        nc.tensor.matmul(po[:], vnat[:, 1 + bb, :], pe[:],
                                         start=False, stop=False)
                    # diagonal blocks j=0..3: queries >= 128*j only
                    for j in range(4):
                        bb = 4 * c + j
                        W = TC - 128 * j
                        qs = slice(c * TC + 128 * j, (c + 1) * TC)
                        st_ = mmps.tile([128, TC], F32, tag="mm")
                        nc.tensor.matmul(st_[:, 0:W],
                                         kT[:, R + bb * 128: R + (bb + 1) * 128],
                                         qT[h][:, qs], start=True, stop=True)
                        # triangular mask on the first 128 cols of this region
                        nc.gpsimd.tensor_add(st_[:, 0:128], st_[:, 0:128], mask_s[:])
                        pe = peb.tile([128, TC], BF16, tag="pe")
                        nc.scalar.activation(pe[:, 0:W], st_[:, 0:W], ACTF.Exp,
                                             scale=SCALING, bias=zeroc[:])
                        last = (j == 3)
                        blocksum(pe[:, 0:W], slice(128 * j, TC))
                        nc.tensor.matmul(po[:, 128 * j:], vnat[:, 1 + bb, :], pe[:, 0:W],
                                         start=False, stop=last)
                    # normalize: oT[h][:, ts] = po * (1/den) broadcast
                    dsum = asb.tile([1, TC], F32, tag="dsum")
                    nc.vector.tensor_add(dsum[:], denp[0][:], denp[1][:])
                    rc = asb.tile([1, TC], BF16, tag="rc")
                    with nc.allow_low_precision(reason="softmax denom in bf16"):
                        nc.vector.reciprocal(rc[:], dsum[:])
                    rb = asb.tile([128, TC], BF16, tag="rb")
                    nc.gpsimd.partition_broadcast(rb[:], rc[:], channels=128)
                    nc.vector.tensor_mul(oT[h][:, ts], po[:], rb[:])

                # ---- output projection for this chunk's 4 t-tiles ----
                for tt in range(4 * c, 4 * c + 4):
                    eng = nc.sync if tt % 2 == 0 else nc.scalar
                    for half in range(2):
                        ob = obb.tile([128, 4, TC], BF16, tag="ob")
                        for jj in range(4):
                            j2 = 4 * half + jj
                            po2 = mmps.tile([128, TC], F32, tag="mm")
                            for h in range(NQH):
                                nc.tensor.matmul(
                                    po2[:], oT[h][:, tt * 128:(tt + 1) * 128],
                                    wo_s[:, h, j2 * TC:(j2 + 1) * TC],
                                    start=(h == 0), stop=(h == NQH - 1))
                            nc.gpsimd.tensor_copy(ob[:, jj, :], po2[:])
                        eng.dma_start(
                            out[tt * 128:(tt + 1) * 128,
                                half * 2048:(half + 1) * 2048], ob[:])

    nc.compile()
    return nc


_NC_CACHE = {}


def _get_nc():
    if "nc" not in _NC_CACHE:
        _NC_CACHE["nc"] = build_nc()
    return _NC_CACHE["nc"]


def _bf(x):
    return np.ascontiguousarray(x.astype(ml_dtypes.bfloat16))


def kernel(**inputs) -> np.ndarray:
    f = lambda k: np.asarray(inputs[k], np.float32)
    hs = f("hidden_states")[0]            # (T, D)
    vk = f("virtual_keys")[0]             # (HKV, R, HD)
    vv = f("virtual_values")[0]
    Wq, Wk, Wv, Wo = f("Wq"), f("Wk"), f("Wv"), f("Wo")
    qnw, knw = f("q_norm_w"), f("k_norm_w")
    lkA, lkB = f("lora_k_A"), f("lora_k_B")
    lvA, lvB = f("lora_v_A"), f("lora_v_B")
    sk = np.float32(np.asarray(inputs["scale_k"]))
    sv = np.float32(np.asarray(inputs["scale_v"]))
    cos, sin = f("cos"), f("sin")         # (T, HD)

    # packed tiles: hsp[c*4+i][p][d8*TC+t] = hs[c*TC+t, (8i+d8)*128+p]
    hsT32 = hs.T.reshape(ND, 128, NTC, TC)          # [dtile, p, c, t]
    hsp = _bf(hsT32.transpose(2, 0, 1, 3)           # [c, dtile, p, t]
              .reshape(NTC, 4, 8, 128, TC)
              .transpose(0, 1, 3, 2, 4)
              .reshape(NTC * 4, 128, 8 * TC))
    # weighted cos/sin for fused (rms*w) + rope:
    #   cw[d,t] = w[d]*cos[t,d]
    #   sw[d,t] = -w[d+64]*sin[t,d]  (d<64);  w[d-64]*sin[t,d]  (d>=64)
    def cw_sw(w):
        cw = (cos.T * w[:, None]).astype(np.float32)
        sw = np.empty((HD, T), np.float32)
        sw[0:64] = -w[64:128, None] * sin.T[0:64]
        sw[64:128] = w[0:64, None] * sin.T[64:128]
        return _bf(cw), _bf(sw)
    cwqh, swqh = cw_sw(qnw)
    cwkh, swkh = cw_sw(knw)
    # constant [128,128] triangular mask: allowed k<=q, else -1e30
    idx = np.arange(128)
    masktri = np.where(idx[:, None] <= idx[None, :], 0.0, -1e30).astype(np.float32)
    ident = np.eye(128, dtype=np.float32)
    lkBs = np.ascontiguousarray(lkB * sk)
    lvBs = np.ascontiguousarray(lvB * sv)

    def wpp_m(m):
        cols = [Wq[:, 512 * m + 128 * p:512 * m + 128 * (p + 1)] for p in range(4)]
        cols.append(Wk[:, 128 * m:128 * (m + 1)])
        cols.append(Wv[:, 128 * m:128 * (m + 1)])
        blocks = [c.reshape(ND, 128, 128).transpose(1, 0, 2).reshape(128, ND * 128)
                  for c in cols]
        return _bf(np.stack(blocks, axis=0))

    in_maps = []
    for m in range(8):
        in_maps.append({
            "hsp": hsp,
            "wpp": wpp_m(m),
            "wo": _bf(Wo[512 * m:512 * (m + 1), :]),
            "cwq": cwqh, "swq": swqh, "cwk": cwkh, "swk": swkh,
            "masktri": masktri,
            "vkT": np.ascontiguousarray(vk[m].T),
            "vvT": np.ascontiguousarray(vv[m].T),
            "lkA": lkA, "lkB": lkBs, "lvA": lvA, "lvB": lvBs,
            "ident": ident,
        })

    nc = _get_nc()
    res = run_bass_kernel_spmd(nc, in_maps, core_ids=list(range(8)))
    acc = res.results[0]["out"].astype(np.float32)
    for m in range(1, 8):
        acc = acc + res.results[m]["out"].astype(np.float32)
    return acc[None]  # (1, T, D)


# revision 17
# speedup vs baseline: 1.1958x; 1.0134x over previous
"""Trainium2 Bass kernel for KVAdapterInjector (Qwen3-style GQA attention with
LoRA-adapted virtual KV prefix).

Sharding: tensor-parallel over heads across 8 cores. Core m gets KV head m and
Q heads 4m..4m+3. Wq/Wk/Wv sharded on output dim, Wo on input dim; partial
outputs (bf16) summed on host.

v2 design notes (cost-model driven):
- All heavy matmuls in bf16 (1.0 cycles/row, immune to the fp32r ap<256
  penalty). PSUM accumulation stays fp32. Measured end-to-end bf16 error
  ~5.5e-3 (budget 2e-2). fp8 was measured at 2.7-5e-2 per stage: rejected.
- PE-row accounting puts the tensor engine at ~370us; all other engines are
  kept under ~150us: softmax denominators stay as ones-matmuls on PE, but
  rms-norm sum/broadcast use gpsimd partition_all_reduce/broadcast (Pool),
  rsqrt = exp(-0.5*ln(x)) on Act (single activation table: ln+exp+square),
  mask-adds and PSUM drains ride Pool, rope elementwise rides DVE in bf16
  (2x mode).
- Causal diagonal blocks are trimmed: block j of a 512-query chunk only
  computes queries >= 128*j, with a constant [128,128] triangular mask tile.
- Chunk-pipelined: proj(c) -> norm/rope(c) -> attention(c) -> outproj(c),
  with PSUM pools sized to exactly 8 banks so phases from adjacent chunks
  overlap across engines.
"""
import sys

sys.path.insert(0, "/opt/trn_rl_repo")

import numpy as np
import ml_dtypes

import concourse.bass as bass
import concourse.mybir as mybir
import concourse.tile as tile
from concourse import bacc
from concourse import bass_isa
from concourse.bass_utils import run_bass_kernel_spmd

F32 = mybir.dt.float32
F32R = mybir.dt.float32r
BF16 = mybir.dt.bfloat16
AX = mybir.AxisListType
ALU = mybir.AluOpType
ACTF = mybir.ActivationFunctionType
RED = bass_isa.ReduceOp

T = 2048
D = 4096
HD = 128
NQH = 4          # q heads per core
R = 64           # virtual tokens
RANK = 16
EPS = 1e-6
SCALING = HD ** -0.5
NTC = 4          # T chunks of 512
TC = 512
ND = D // 128    # 32 contraction tiles
NKB = T // 128   # 16 key blocks (real)


def build_nc():
    nc = bacc.Bacc(None, target_bir_lowering=False, debug=False)

    # ---- DRAM I/O (bf16 activations/weights prepared on host) ----
    wpp = nc.dram_tensor("wpp", (6, 128, ND * 128), BF16, kind="ExternalInput")
    hsp = nc.dram_tensor("hsp", (NTC * 4, 128, 8 * TC), BF16, kind="ExternalInput")
    wo = nc.dram_tensor("wo", (NQH * HD, D), BF16, kind="ExternalInput")
    cwq = nc.dram_tensor("cwq", (HD, T), BF16, kind="ExternalInput")
    swq = nc.dram_tensor("swq", (HD, T), BF16, kind="ExternalInput")
    cwk = nc.dram_tensor("cwk", (HD, T), BF16, kind="ExternalInput")
    swk = nc.dram_tensor("swk", (HD, T), BF16, kind="ExternalInput")
    masktri = nc.dram_tensor("masktri", (128, 128), F32, kind="ExternalInput")
    vkT = nc.dram_tensor("vkT", (HD, R), F32, kind="ExternalInput")
    vvT = nc.dram_tensor("vvT", (HD, R), F32, kind="ExternalInput")
    lkA = nc.dram_tensor("lkA", (HD, RANK), F32, kind="ExternalInput")
    lkB = nc.dram_tensor("lkB", (RANK, HD), F32, kind="ExternalInput")  # pre-scaled
    lvA = nc.dram_tensor("lvA", (HD, RANK), F32, kind="ExternalInput")
    lvB = nc.dram_tensor("lvB", (RANK, HD), F32, kind="ExternalInput")  # pre-scaled
    ident = nc.dram_tensor("ident", (128, 128), F32, kind="ExternalInput")
    out = nc.dram_tensor("out", (T, D), BF16, kind="ExternalOutput")

    r = lambda ap: ap.bitcast(F32R)

    from contextlib import ExitStack
    with tile.TileContext(nc) as tc, ExitStack() as est:
        cp = est.enter_context(tc.tile_pool(name="consts", bufs=1))
        pp = est.enter_context(tc.tile_pool(name="persist", bufs=1))

        # pin the Act table that serves square+ln+exp, so the auto-insertion
        # pass doesn't thrash between natural_log and exp tables
        from concourse.hw_specs import get_activation_tables
        _tables = list(get_activation_tables(nc.m.arch).keys())
        _atl = mybir.InstLoadActFuncSet(
            name=nc.get_next_instruction_name(), ins=[], outs=[],
            act_func_set_id=_tables.index("natural_log_exp_and_others"))
        _atl.engine = mybir.EngineType.Activation
        nc.scalar.add_instruction(_atl)

        # ---- small consts ----
        onesb = cp.tile([128, 1], BF16)
        nc.vector.memset(onesb[:], 1.0)
        epsc = cp.tile([128, 1], F32)
        nc.vector.memset(epsc[:], EPS)
        zeroc = cp.tile([128, 1], F32)
        nc.vector.memset(zeroc[:], 0.0)
        mask_s = cp.tile([128, 128], F32)
        nc.scalar.dma_start(mask_s[:], masktri[:])

        # ---- persistent activations ----
        # qT[h]: rope'd queries, [HD, T] bf16; aliased as oT (attention output)
        qT = [pp.tile([HD, T], BF16, tag=f"qT{h}", name=f"qT{h}") for h in range(NQH)]
        oT = qT
        kT = pp.tile([HD, R + T], BF16)           # cols 0:64 = adapted virtual keys
        vnat = pp.tile([128, NKB + 1, 128], BF16)  # block 0 = virtual values (rows 0:64)

        # ---- rope/norm consts (weighted cos/sin) ----
        cwq_s = cp.tile([HD, T], BF16)
        swq_s = cp.tile([HD, T], BF16)
        cwk_s = cp.tile([HD, T], BF16)
        swk_s = cp.tile([HD, T], BF16)

        # ---- weights in SBUF ----
        wqkv_s = cp.tile([128, 6, ND, 128], BF16)  # passes q0..q3, k, v
        wo_s = cp.tile([128, NQH, D], BF16)

        # ================= Phase 0: LoRA-adapt virtual KV (tiny) =================
        lsb = cp
        with tc.tile_pool(name="lora_ps", bufs=1, space="PSUM") as lps:
            vkT_s = lsb.tile([HD, R], F32R)
            vvT_s = lsb.tile([HD, R], F32R)
            lkA_s = lsb.tile([HD, RANK], F32R)
            lkB_s = lsb.tile([RANK, HD], F32R)
            lvA_s = lsb.tile([HD, RANK], F32R)
            lvB_s = lsb.tile([RANK, HD], F32R)
            ident_s = lsb.tile([128, 128], F32R)
            nc.scalar.dma_start(vkT_s[:], r(vkT[:]))
            nc.scalar.dma_start(vvT_s[:], r(vvT[:]))
            nc.scalar.dma_start(lkA_s[:], r(lkA[:]))
            nc.scalar.dma_start(lkB_s[:], r(lkB[:]))
            nc.scalar.dma_start(lvA_s[:], r(lvA[:]))
            nc.scalar.dma_start(lvB_s[:], r(lvB[:]))
            nc.scalar.dma_start(ident_s[:], r(ident[:]))
            # keys: kT[:, 0:64] = vkT + Bk^T Ak^T vkT  (Bk pre-scaled)
            t1 = lps.tile([RANK, R], F32, tag="l1")
            nc.tensor.matmul(t1[:], lkA_s[:], vkT_s[:], start=True, stop=True)
            t1s = lsb.tile([RANK, R], F32R)
            nc.scalar.copy(t1s[:], t1[:])
            t2 = lps.tile([HD, R], F32, tag="l2")
            nc.tensor.matmul(t2[:], lkB_s[:], t1s[:], start=True, stop=True)
            nc.vector.tensor_add(kT[:, 0:R], vkT_s[:].bitcast(F32), t2[:])
            # values
            u1 = lps.tile([RANK, R], F32, tag="l1")
            nc.tensor.matmul(u1[:], lvA_s[:], vvT_s[:], start=True, stop=True)
            u1s = lsb.tile([RANK, R], F32R)
            nc.scalar.copy(u1s[:], u1[:])
            u2 = lps.tile([HD, R], F32, tag="l2")
            nc.tensor.matmul(u2[:], lvB_s[:], u1s[:], start=True, stop=True)
            vvirt = lsb.tile([HD, R], F32R)
            with nc.allow_low_precision(reason="f32r same width as f32"):
                nc.vector.tensor_add(vvirt[:], vvT_s[:].bitcast(F32), u2[:])
            # transpose virtual values to natural layout -> vnat[0:64, 0, :]
            vtp = lps.tile([R, HD], F32R, tag="l3")
            nc.tensor.transpose(vtp[:], vvirt[:], ident_s[:])
            nc.gpsimd.tensor_copy(vnat[0:R, 0, :], vtp[:].bitcast(F32))

        # ---- weight / rope-const loads, ordered for earliest PE start ----
        pm = lambda ap: ap.rearrange("(n p) c -> p n c", p=128)
        nc.sync.dma_start(wqkv_s[:, 0, :, :], wpp[0])

        # ================= main chunk pipeline =================
        with tc.tile_pool(name="proj_ps", bufs=2, space="PSUM") as prps, \
             tc.tile_pool(name="mm_ps", bufs=4, space="PSUM") as mmps, \
             tc.tile_pool(name="b2k_ps", bufs=2, space="PSUM") as b2ps, \
             tc.tile_pool(name="hs_sb", bufs=1) as hsb, \
             tc.tile_pool(name="nrm_sb", bufs=2) as nsb, \
             tc.tile_pool(name="pe_sb", bufs=7) as peb, \
             tc.tile_pool(name="at_sb", bufs=2) as asb, \
             tc.tile_pool(name="ob_sb", bufs=2) as obb:
            def new_hs(c_):
                return [hsb.tile([128, 8, TC], BF16, tag=f"hs{i}",
                                 name=f"hs{c_}_{i}") for i in range(4)]
            def load_hs(tiles_, c_, engs=None):
                for i in range(4):
                    eng = nc.sync if engs is None else engs[i]
                    eng.dma_start(tiles_[i][:], hsp[4 * c_ + i])
            hs_tiles = {0: new_hs(0)}
            load_hs(hs_tiles[0], 0, engs=[nc.sync, nc.scalar, nc.sync, nc.scalar])
            nc.sync.dma_start(wqkv_s[:, 1, :, :], wpp[1])
            nc.sync.dma_start(wqkv_s[:, 2, :, :], wpp[2])
            nc.sync.dma_start(cwq_s[:], cwq[:])
            nc.sync.dma_start(swq_s[:], swq[:])
            nc.sync.dma_start(wqkv_s[:, 3, :, :], wpp[3])
            nc.sync.dma_start(wqkv_s[:, 4, :, :], wpp[4])
            nc.sync.dma_start(wqkv_s[:, 5, :, :], wpp[5])
            nc.sync.dma_start(cwk_s[:], cwk[:])
            nc.sync.dma_start(swk_s[:], swk[:])
            nc.sync.dma_start(wo_s[:], pm(wo[:, :]))
            for c in range(NTC):
                ts = slice(c * TC, (c + 1) * TC)
                hs_c = hs_tiles.pop(c)

                # ---- projections: 5 passes (q0..q3, k), each one accumulator ----
                for p in range(NQH + 1):
                    pacc = prps.tile([128, TC], F32, tag="pacc")
                    wslice = wqkv_s[:, p, :, :]
                    for d in range(ND):
                        nc.tensor.matmul(pacc[:], wslice[:, d, :],
                                         hs_c[d // 8][:, d % 8, :],
                                         start=(d == 0), stop=(d == ND - 1))
                    # ---- rms-norm + rope on this pass's PSUM ----
                    isq = p < NQH
                    cw = cwq_s if isq else cwk_s
                    sw = swq_s if isq else swk_s
                    dst = qT[p][:, ts] if isq else kT[:, R + c * TC: R + (c + 1) * TC]
                    sq = nsb.tile([HD, TC], BF16, tag="sq")
                    nc.gpsimd.tensor_mul(sq[:], pacc[:], pacc[:])
                    ssum = nsb.tile([HD, TC], BF16, tag="ssum")
                    nc.gpsimd.partition_all_reduce(ssum[:], sq[:], channels=128,
                                                   reduce_op=RED.add)
                    lns = nsb.tile([HD, TC], F32, tag="lns")
                    nc.scalar.activation(lns[:], ssum[:], ACTF.Ln,
                                         scale=1.0 / HD, bias=epsc[:])
                    rinv = nsb.tile([HD, TC], BF16, tag="rinv")
                    nc.scalar.activation(rinv[:], lns[:], ACTF.Exp, scale=-0.5,
                                         bias=zeroc[:])
                    xn = nsb.tile([HD, TC], BF16, tag="xn")
                    nc.vector.tensor_mul(xn[:], pacc[:], rinv[:])
                    t1 = nsb.tile([HD, TC], BF16, tag="t1")
                    nc.vector.tensor_mul(t1[:], xn[:], cw[:, ts])
                    t2 = nsb.tile([HD, TC], BF16, tag="t2")
                    nc.vector.tensor_mul(t2[0:64, :], xn[64:128, :], sw[0:64, ts])
                    nc.vector.tensor_mul(t2[64:128, :], xn[0:64, :], sw[64:128, ts])
                    nc.vector.tensor_add(dst, t1[:], t2[:])

                # ---- V in natural layout: stationary = hs t-slices ----
                vacc = b2ps.tile([128, 4, 128], F32, tag="b2k")
                for tt in range(4):
                    for d in range(ND):
                        nc.tensor.matmul(vacc[:, tt, :],
                                         hs_c[d // 8][:, d % 8,
                                                      tt * 128:(tt + 1) * 128],
                                         wqkv_s[:, 5, d, :],
                                         start=(d == 0), stop=(d == ND - 1))
                nc.gpsimd.tensor_copy(vnat[:, 1 + 4 * c: 5 + 4 * c, :], vacc[:])

                # prefetch next chunk's hidden states (after last hs_c reader issued)
                if c + 1 < NTC:
                    hs_tiles[c + 1] = new_hs(c + 1)
                    load_hs(hs_tiles[c + 1], c + 1)

                # ---- attention for this chunk's queries, interleaved with
                # ---- the previous chunk's output projection (fills PE while
                # ---- attention is Act-bound)
                def outproj_tile(tt):
                    eng = nc.sync if tt % 2 == 0 else nc.scalar
                    for half in range(2):
                        ob = obb.tile([128, 4, TC], BF16, tag="ob")
                        for jj in range(4):
                            j2 = 4 * half + jj
                            po2 = mmps.tile([128, TC], F32, tag="mm")
                            for h2 in range(NQH):
                                nc.tensor.matmul(
                                    po2[:], oT[h2][:, tt * 128:(tt + 1) * 128],
                                    wo_s[:, h2, j2 * TC:(j2 + 1) * TC],
                                    start=(h2 == 0), stop=(h2 == NQH - 1))
                            dr = nc.gpsimd if j2 % 2 == 0 else nc.vector
                            dr.tensor_copy(ob[:, jj, :], po2[:])
                        eng.dma_start(
                            out[tt * 128:(tt + 1) * 128,
                                half * 2048:(half + 1) * 2048], ob[:])

                for h in range(NQH):
                    # two parity accumulators halve the serial add chain
                    denp = [asb.tile([1, TC], F32, tag="denE", bufs=2, name="denE"),
                            asb.tile([1, TC], F32, tag="denO", bufs=2, name="denO")]
                    blk_i = [0]
                    po = b2ps.tile([128, TC], F32, tag="b2k")

                    def blocksum(pe_ap, wslc):
                        # denominator accumulation off-PE: partition-reduce
                        # on Pool, then row-add into den parity acc on DVE
                        i = blk_i[0]; blk_i[0] += 1
                        den_ = denp[i % 2]
                        ps_ = peb.tile([128, TC], BF16, tag="ps", bufs=4)
                        rows = pe_ap.shape[0]
                        W_ = pe_ap.shape[-1]
                        nc.gpsimd.partition_all_reduce(
                            ps_[0:rows, 0:W_], pe_ap, channels=rows,
                            reduce_op=RED.add)
                        if i < 2:
                            nc.vector.tensor_copy(den_[:], ps_[0:1, 0:W_])
                        else:
                            nc.vector.tensor_add(den_[:, wslc], den_[:, wslc],
                                                 ps_[0:1, 0:W_])

                    # virtual block (full width)
                    st_ = mmps.tile([128, TC], F32, tag="mm")
                    nc.tensor.matmul(st_[0:R, :], kT[:, 0:R], qT[h][:, ts],
                                     start=True, stop=True)
                    pe = peb.tile([128, TC], BF16, tag="pe")
                    nc.scalar.activation(pe[0:R, :], st_[0:R, :], ACTF.Exp,
                                         scale=SCALING, bias=zeroc[0:R, :])
                    blocksum(pe[0:R, :], slice(0, TC))
                    nc.tensor.matmul(po[:], vnat[0:R, 0, :], pe[0:R, :],
                                     start=True, stop=False)
                    # full (past) key blocks
                    for bb in range(4 * c):
                        st_ = mmps.tile([128, TC], F32, tag="mm")
                        nc.tensor.matmul(st_[:], kT[:, R + bb * 128: R + (bb + 1) * 128],
                                         qT[h][:, ts], start=True, stop=True)
                        pe = peb.tile([128, TC], BF16, tag="pe")
                        nc.scalar.activation(pe[:], st_[:], ACTF.Exp,
                                             scale=SCALING, bias=zeroc[:])
                        blocksum(pe[:], slice(0, TC))
                        nc.tensor.matmul(po[:], vnat[:, 1 + bb, :], pe[:],
                                         start=False, stop=False)
                    # diagonal blocks j=0..3: queries >= 128*j only
                    for j in range(4):
                        bb = 4 * c + j
                        W = TC - 128 * j
                        qs = slice(c * TC + 128 * j, (c + 1) * TC)
                        st_ = mmps.tile([128, TC], F32, tag="mm")
                        nc.tensor.matmul(st_[:, 0:W],
                                         kT[:, R + bb * 128: R + (bb + 1) * 128],
                                         qT[h][:, qs], start=True, stop=True)
                        # triangular mask on the first 128 cols of this region
                        nc.gpsimd.tensor_add(st_[:, 0:128], st_[:, 0:128], mask_s[:])
                        pe = peb.tile([128, TC], BF16, tag="pe")
                        nc.scalar.activation(pe[:, 0:W], st_[:, 0:W], ACTF.Exp,
                                             scale=SCALING, bias=zeroc[:])
                        last = (j == 3)
                        blocksum(pe[:, 0:W], slice(128 * j, TC))
                        nc.tensor.matmul(po[:, 128 * j:], vnat[:, 1 + bb, :], pe[:, 0:W],
                                         start=False, stop=last)
                    # normalize: oT[h][:, ts] = po * (1/den) broadcast
                    dsum = asb.tile([1, TC], F32, tag="dsum")
                    nc.vector.tensor_add(dsum[:], denp[0][:], denp[1][:])
                    rc = asb.tile([1, TC], BF16, tag="rc")
                    with nc.allow_low_precision(reason="softmax denom in bf16"):
                        nc.vector.reciprocal(rc[:], dsum[:])
                    rb = asb.tile([128, TC], BF16, tag="rb")
                    nc.gpsimd.partition_broadcast(rb[:], rc[:], channels=128)
                    nc.vector.tensor_mul(oT[h][:, ts], po[:], rb[:])
                    if c > 0:
                        outproj_tile(4 * (c - 1) + h)

                if c == NTC - 1:
                    for tt in range(4 * c, 4 * c + 4):
                        outproj_tile(tt)


    nc.compile()
    return nc


_NC_CACHE = {}


def _get_nc():
    if "nc" not in _NC_CACHE:
        _NC_CACHE["nc"] = build_nc()
    return _NC_CACHE["nc"]


def _bf(x):
    return np.ascontiguousarray(x.astype(ml_dtypes.bfloat16))


def kernel(**inputs) -> np.ndarray:
    f = lambda k: np.asarray(inputs[k], np.float32)
    hs = f("hidden_states")[0]            # (T, D)
    vk = f("virtual_keys")[0]             # (HKV, R, HD)
    vv = f("virtual_values")[0]
    Wq, Wk, Wv, Wo = f("Wq"), f("Wk"), f("Wv"), f("Wo")
    qnw, knw = f("q_norm_w"), f("k_norm_w")
    lkA, lkB = f("lora_k_A"), f("lora_k_B")
    lvA, lvB = f("lora_v_A"), f("lora_v_B")
    sk = np.float32(np.asarray(inputs["scale_k"]))
    sv = np.float32(np.asarray(inputs["scale_v"]))
    cos, sin = f("cos"), f("sin")         # (T, HD)

    # packed tiles: hsp[c*4+i][p][d8*TC+t] = hs[c*TC+t, (8i+d8)*128+p]
    hsT32 = hs.T.reshape(ND, 128, NTC, TC)          # [dtile, p, c, t]
    hsp = _bf(hsT32.transpose(2, 0, 1, 3)           # [c, dtile, p, t]
              .reshape(NTC, 4, 8, 128, TC)
              .transpose(0, 1, 3, 2, 4)
              .reshape(NTC * 4, 128, 8 * TC))
    # weighted cos/sin for fused (rms*w) + rope:
    #   cw[d,t] = w[d]*cos[t,d]
    #   sw[d,t] = -w[d+64]*sin[t,d]  (d<64);  w[d-64]*sin[t,d]  (d>=64)
    def cw_sw(w):
        cw = (cos.T * w[:, None]).astype(np.float32)
        sw = np.empty((HD, T), np.float32)
        sw[0:64] = -w[64:128, None] * sin.T[0:64]
        sw[64:128] = w[0:64, None] * sin.T[64:128]
        return _bf(cw), _bf(sw)
    cwqh, swqh = cw_sw(qnw)
    cwkh, swkh = cw_sw(knw)
    # constant [128,128] triangular mask: allowed k<=q, else -1e30
    idx = np.arange(128)
    masktri = np.where(idx[:, None] <= idx[None, :], 0.0, -1e30).astype(np.float32)
    ident = np.eye(128, dtype=np.float32)
    lkBs = np.ascontiguousarray(lkB * sk)
    lvBs = np.ascontiguousarray(lvB * sv)

    def wpp_m(m):
        cols = [Wq[:, 512 * m + 128 * p:512 * m + 128 * (p + 1)] for p in range(4)]
        cols.append(Wk[:, 128 * m:128 * (m + 1)])
        cols.append(Wv[:, 128 * m:128 * (m + 1)])
        blocks = [c.reshape(ND, 128, 128).transpose(1, 0, 2).reshape(128, ND * 128)
                  for c in cols]
        return _bf(np.stack(blocks, axis=0))

    in_maps = []
    for m in range(8):
        in_maps.append({
            "hsp": hsp,
            "wpp": wpp_m(m),
            "wo": _bf(Wo[512 * m:512 * (m + 1), :]),
            "cwq": cwqh, "swq": swqh, "cwk": cwkh, "swk": swkh,
            "masktri": masktri,
            "vkT": np.ascontiguousarray(vk[m].T),
            "vvT": np.ascontiguousarray(vv[m].T),
            "lkA": lkA, "lkB": lkBs, "lvA": lvA, "lvB": lvBs,
            "ident": ident,
        })

    nc = _get_nc()
    res = run_bass_kernel_spmd(nc, in_maps, core_ids=list(range(8)))
    acc = res.results[0]["out"].astype(np.float32)
    for m in range(1, 8):
        acc = acc + res.results[m]["out"].astype(np.float32)
    return acc[None]  # (1, T, D)


# revision 18
# speedup vs baseline: 1.2416x; 1.0383x over previous
"""Trainium2 Bass kernel for KVAdapterInjector (Qwen3-style GQA attention with
LoRA-adapted virtual KV prefix).

Sharding: tensor-parallel over heads across 8 cores. Core m gets KV head m and
Q heads 4m..4m+3. Wq/Wk/Wv sharded on output dim, Wo on input dim; partial
outputs (bf16) summed on host.

v2 design notes (cost-model driven):
- All heavy matmuls in bf16 (1.0 cycles/row, immune to the fp32r ap<256
  penalty). PSUM accumulation stays fp32. Measured end-to-end bf16 error
  ~5.5e-3 (budget 2e-2). fp8 was measured at 2.7-5e-2 per stage: rejected.
- PE-row accounting puts the tensor engine at ~370us; all other engines are
  kept under ~150us: softmax denominators stay as ones-matmuls on PE, but
  rms-norm sum/broadcast use gpsimd partition_all_reduce/broadcast (Pool),
  rsqrt = exp(-0.5*ln(x)) on Act (single activation table: ln+exp+square),
  mask-adds and PSUM drains ride Pool, rope elementwise rides DVE in bf16
  (2x mode).
- Causal diagonal blocks are trimmed: block j of a 512-query chunk only
  computes queries >= 128*j, with a constant [128,128] triangular mask tile.
- Chunk-pipelined: proj(c) -> norm/rope(c) -> attention(c) -> outproj(c),
  with PSUM pools sized to exactly 8 banks so phases from adjacent chunks
  overlap across engines.
"""
import sys

sys.path.insert(0, "/opt/trn_rl_repo")

import numpy as np
import ml_dtypes

import concourse.bass as bass
import concourse.mybir as mybir
import concourse.tile as tile
from concourse import bacc
from concourse import bass_isa
from concourse.bass_utils import run_bass_kernel_spmd

F32 = mybir.dt.float32
F32R = mybir.dt.float32r
BF16 = mybir.dt.bfloat16
AX = mybir.AxisListType
ALU = mybir.AluOpType
ACTF = mybir.ActivationFunctionType
RED = bass_isa.ReduceOp

T = 2048
D = 4096
HD = 128
NQH = 4          # q heads per core
R = 64           # virtual tokens
RANK = 16
EPS = 1e-6
SCALING = HD ** -0.5
NTC = 4          # T chunks of 512
TC = 512
ND = D // 128    # 32 contraction tiles
NKB = T // 128   # 16 key blocks (real)


def build_nc():
    nc = bacc.Bacc(None, target_bir_lowering=False, debug=False)

    # ---- DRAM I/O (bf16 activations/weights prepared on host) ----
    wpp = nc.dram_tensor("wpp", (6, 128, ND * 128), BF16, kind="ExternalInput")
    hsp = nc.dram_tensor("hsp", (NTC * 4, 128, 8 * TC), BF16, kind="ExternalInput")
    wo = nc.dram_tensor("wo", (NQH * HD, D), BF16, kind="ExternalInput")
    cwq = nc.dram_tensor("cwq", (HD, T), BF16, kind="ExternalInput")
    swq = nc.dram_tensor("swq", (HD, T), BF16, kind="ExternalInput")
    cwk = nc.dram_tensor("cwk", (HD, T), BF16, kind="ExternalInput")
    swk = nc.dram_tensor("swk", (HD, T), BF16, kind="ExternalInput")
    masktri = nc.dram_tensor("masktri", (128, 128), F32, kind="ExternalInput")
    kvirt = nc.dram_tensor("kvirt", (HD, R), BF16, kind="ExternalInput")
    vvirt = nc.dram_tensor("vvirt", (R, HD), BF16, kind="ExternalInput")
    out = nc.dram_tensor("out", (T, D), BF16, kind="ExternalOutput")

    from contextlib import ExitStack
    with tile.TileContext(nc) as tc, ExitStack() as est:
        cp = est.enter_context(tc.tile_pool(name="consts", bufs=1))
        pp = est.enter_context(tc.tile_pool(name="persist", bufs=1))

        # pin the Act table that serves square+ln+exp, so the auto-insertion
        # pass doesn't thrash between natural_log and exp tables
        from concourse.hw_specs import get_activation_tables
        _tables = list(get_activation_tables(nc.m.arch).keys())
        _atl = mybir.InstLoadActFuncSet(
            name=nc.get_next_instruction_name(), ins=[], outs=[],
            act_func_set_id=_tables.index("natural_log_exp_and_others"))
        _atl.engine = mybir.EngineType.Activation
        nc.scalar.add_instruction(_atl)

        # ---- small consts ----
        onesb = cp.tile([128, 1], BF16)
        nc.vector.memset(onesb[:], 1.0)
        epsc = cp.tile([128, 1], F32)
        nc.vector.memset(epsc[:], EPS)
        zeroc = cp.tile([128, 1], F32)
        nc.vector.memset(zeroc[:], 0.0)
        mask_s = cp.tile([128, 128], F32)
        nc.scalar.dma_start(mask_s[:], masktri[:])

        # ---- persistent activations ----
        # qT[h]: rope'd queries, [HD, T] bf16; aliased as oT (attention output)
        qT = [pp.tile([HD, T], BF16, tag=f"qT{h}", name=f"qT{h}") for h in range(NQH)]
        oT = qT
        kT = pp.tile([HD, R + T], BF16)           # cols 0:64 = adapted virtual keys
        vnat = pp.tile([128, NKB + 1, 128], BF16)  # block 0 = virtual values (rows 0:64)

        # ---- rope/norm consts (weighted cos/sin) ----
        cwq_s = cp.tile([HD, T], BF16)
        swq_s = cp.tile([HD, T], BF16)
        cwk_s = cp.tile([HD, T], BF16)
        swk_s = cp.tile([HD, T], BF16)

        # ---- weights in SBUF ----
        wqkv_s = cp.tile([128, 6, ND, 128], BF16)  # passes q0..q3, k, v
        wo_s = cp.tile([128, NQH, D], BF16)

        # ---- LoRA-adapted virtual KV is computed on host (2 MFLOP) ----
        nc.scalar.dma_start(kT[:, 0:R], kvirt[:])
        nc.scalar.dma_start(vnat[0:R, 0, :], vvirt[:])

        # ---- weight / rope-const loads, ordered for earliest PE start ----
        pm = lambda ap: ap.rearrange("(n p) c -> p n c", p=128)
        nc.sync.dma_start(wqkv_s[:, 0, :, :], wpp[0])

        # ================= main chunk pipeline =================
        with tc.tile_pool(name="proj_ps", bufs=2, space="PSUM") as prps, \
             tc.tile_pool(name="mm_ps", bufs=4, space="PSUM") as mmps, \
             tc.tile_pool(name="b2k_ps", bufs=2, space="PSUM") as b2ps, \
             tc.tile_pool(name="hs_sb", bufs=1) as hsb, \
             tc.tile_pool(name="nrm_sb", bufs=2) as nsb, \
             tc.tile_pool(name="pe_sb", bufs=7) as peb, \
             tc.tile_pool(name="at_sb", bufs=2) as asb, \
             tc.tile_pool(name="ob_sb", bufs=2) as obb:
            def new_hs(c_):
                return [hsb.tile([128, 8, TC], BF16, tag=f"hs{i}",
                                 name=f"hs{c_}_{i}") for i in range(4)]
            def load_hs(tiles_, c_, engs=None):
                for i in range(4):
                    eng = nc.sync if engs is None else engs[i]
                    eng.dma_start(tiles_[i][:], hsp[4 * c_ + i])
            hs_tiles = {0: new_hs(0)}
            load_hs(hs_tiles[0], 0, engs=[nc.sync, nc.scalar, nc.sync, nc.scalar])
            nc.sync.dma_start(wqkv_s[:, 1, :, :], wpp[1])
            nc.sync.dma_start(wqkv_s[:, 2, :, :], wpp[2])
            nc.sync.dma_start(cwq_s[:], cwq[:])
            nc.sync.dma_start(swq_s[:], swq[:])
            nc.sync.dma_start(wqkv_s[:, 3, :, :], wpp[3])
            nc.sync.dma_start(wqkv_s[:, 4, :, :], wpp[4])
            nc.sync.dma_start(wqkv_s[:, 5, :, :], wpp[5])
            nc.sync.dma_start(cwk_s[:], cwk[:])
            nc.sync.dma_start(swk_s[:], swk[:])
            nc.sync.dma_start(wo_s[:], pm(wo[:, :]))
            for c in range(NTC):
                ts = slice(c * TC, (c + 1) * TC)
                hs_c = hs_tiles.pop(c)

                # ---- projections: 5 passes (q0..q3, k), each one accumulator ----
                for p in range(NQH + 1):
                    pacc = prps.tile([128, TC], F32, tag="pacc")
                    wslice = wqkv_s[:, p, :, :]
                    for d in range(ND):
                        nc.tensor.matmul(pacc[:], wslice[:, d, :],
                                         hs_c[d // 8][:, d % 8, :],
                                         start=(d == 0), stop=(d == ND - 1))
                    # ---- rms-norm + rope on this pass's PSUM ----
                    isq = p < NQH
                    cw = cwq_s if isq else cwk_s
                    sw = swq_s if isq else swk_s
                    dst = qT[p][:, ts] if isq else kT[:, R + c * TC: R + (c + 1) * TC]
                    sq = nsb.tile([HD, TC], BF16, tag="sq")
                    nc.gpsimd.tensor_mul(sq[:], pacc[:], pacc[:])
                    ssum = nsb.tile([HD, TC], BF16, tag="ssum")
                    nc.gpsimd.partition_all_reduce(ssum[:], sq[:], channels=128,
                                                   reduce_op=RED.add)
                    lns = nsb.tile([HD, TC], F32, tag="lns")
                    nc.scalar.activation(lns[:], ssum[:], ACTF.Ln,
                                         scale=1.0 / HD, bias=epsc[:])
                    rinv = nsb.tile([HD, TC], BF16, tag="rinv")
                    nc.scalar.activation(rinv[:], lns[:], ACTF.Exp, scale=-0.5,
                                         bias=zeroc[:])
                    xn = nsb.tile([HD, TC], BF16, tag="xn")
                    nc.vector.tensor_mul(xn[:], pacc[:], rinv[:])
                    t1 = nsb.tile([HD, TC], BF16, tag="t1")
                    nc.vector.tensor_mul(t1[:], xn[:], cw[:, ts])
                    t2 = nsb.tile([HD, TC], BF16, tag="t2")
                    nc.vector.tensor_mul(t2[0:64, :], xn[64:128, :], sw[0:64, ts])
                    nc.vector.tensor_mul(t2[64:128, :], xn[0:64, :], sw[64:128, ts])
                    nc.vector.tensor_add(dst, t1[:], t2[:])

                # ---- V in natural layout: stationary = hs t-slices ----
                vacc = b2ps.tile([128, 4, 128], F32, tag="b2k")
                for tt in range(4):
                    for d in range(ND):
                        nc.tensor.matmul(vacc[:, tt, :],
                                         hs_c[d // 8][:, d % 8,
                                                      tt * 128:(tt + 1) * 128],
                                         wqkv_s[:, 5, d, :],
                                         start=(d == 0), stop=(d == ND - 1))
                nc.gpsimd.tensor_copy(vnat[:, 1 + 4 * c: 5 + 4 * c, :], vacc[:])

                # prefetch next chunk's hidden states (after last hs_c reader issued)
                if c + 1 < NTC:
                    hs_tiles[c + 1] = new_hs(c + 1)
                    load_hs(hs_tiles[c + 1], c + 1)

                # ---- attention for this chunk's queries, interleaved with
                # ---- the previous chunk's output projection (fills PE while
                # ---- attention is Act-bound)
                op_state = {"u": 0, "ob": None, "base": 4 * (c - 1)}

                def outproj_unit():
                    # emit one (tt, j2) unit of the previous chunk's output
                    # projection; group 4 units per ob tile + one DMA
                    u = op_state["u"]
                    if u >= 32 * (1 if c > 0 else 0):
                        return False
                    op_state["u"] = u + 1
                    tt = op_state["base"] + u // 8
                    half = (u // 4) % 2
                    jj = u % 4
                    j2 = 4 * half + jj
                    if jj == 0:
                        op_state["ob"] = obb.tile([128, 4, TC], BF16, tag="ob",
                                                  name=f"ob{c}_{u}")
                    ob = op_state["ob"]
                    po2 = mmps.tile([128, TC], F32, tag="mm")
                    for h2 in range(NQH):
                        nc.tensor.matmul(
                            po2[:], oT[h2][:, tt * 128:(tt + 1) * 128],
                            wo_s[:, h2, j2 * TC:(j2 + 1) * TC],
                            start=(h2 == 0), stop=(h2 == NQH - 1))
                    dr = nc.gpsimd if j2 % 2 == 0 else nc.vector
                    dr.tensor_copy(ob[:, jj, :], po2[:])
                    if jj == 3:
                        eng = nc.sync if tt % 2 == 0 else nc.scalar
                        eng.dma_start(
                            out[tt * 128:(tt + 1) * 128,
                                half * 2048:(half + 1) * 2048], ob[:])
                    return True

                for h in range(NQH):
                    # two parity accumulators halve the serial add chain
                    denp = [asb.tile([1, TC], F32, tag="denE", bufs=2, name="denE"),
                            asb.tile([1, TC], F32, tag="denO", bufs=2, name="denO")]
                    blk_i = [0]
                    po = b2ps.tile([128, TC], F32, tag="b2k")

                    def blocksum(pe_ap, wslc):
                        # denominator accumulation off-PE: partition-reduce
                        # on Pool, then row-add into den parity acc on DVE
                        i = blk_i[0]; blk_i[0] += 1
                        den_ = denp[i % 2]
                        ps_ = peb.tile([128, TC], BF16, tag="ps", bufs=4)
                        rows = pe_ap.shape[0]
                        W_ = pe_ap.shape[-1]
                        nc.gpsimd.partition_all_reduce(
                            ps_[0:rows, 0:W_], pe_ap, channels=rows,
                            reduce_op=RED.add)
                        if i < 2:
                            nc.vector.tensor_copy(den_[:], ps_[0:1, 0:W_])
                        else:
                            nc.vector.tensor_add(den_[:, wslc], den_[:, wslc],
                                                 ps_[0:1, 0:W_])

                    # virtual block (full width)
                    st_ = mmps.tile([128, TC], F32, tag="mm")
                    nc.tensor.matmul(st_[0:R, :], kT[:, 0:R], qT[h][:, ts],
                                     start=True, stop=True)
                    pe = peb.tile([128, TC], BF16, tag="pe")
                    nc.scalar.activation(pe[0:R, :], st_[0:R, :], ACTF.Exp,
                                         scale=SCALING, bias=zeroc[0:R, :])
                    blocksum(pe[0:R, :], slice(0, TC))
                    nc.tensor.matmul(po[:], vnat[0:R, 0, :], pe[0:R, :],
                                     start=True, stop=False)
                    # full (past) key blocks
                    for bb in range(4 * c):
                        st_ = mmps.tile([128, TC], F32, tag="mm")
                        nc.tensor.matmul(st_[:], kT[:, R + bb * 128: R + (bb + 1) * 128],
                                         qT[h][:, ts], start=True, stop=True)
                        pe = peb.tile([128, TC], BF16, tag="pe")
                        nc.scalar.activation(pe[:], st_[:], ACTF.Exp,
                                             scale=SCALING, bias=zeroc[:])
                        blocksum(pe[:], slice(0, TC))
                        nc.tensor.matmul(po[:], vnat[:, 1 + bb, :], pe[:],
                                         start=False, stop=False)
                        if bb % 2 == 1:
                            outproj_unit()
                    # diagonal blocks j=0..3: queries >= 128*j only
                    for j in range(4):
                        bb = 4 * c + j
                        W = TC - 128 * j
                        qs = slice(c * TC + 128 * j, (c + 1) * TC)
                        st_ = mmps.tile([128, TC], F32, tag="mm")
                        nc.tensor.matmul(st_[:, 0:W],
                                         kT[:, R + bb * 128: R + (bb + 1) * 128],
                                         qT[h][:, qs], start=True, stop=True)
                        # triangular mask on the first 128 cols of this region
                        nc.gpsimd.tensor_add(st_[:, 0:128], st_[:, 0:128], mask_s[:])
                        pe = peb.tile([128, TC], BF16, tag="pe")
                        nc.scalar.activation(pe[:, 0:W], st_[:, 0:W], ACTF.Exp,
                                             scale=SCALING, bias=zeroc[:])
                        last = (j == 3)
                        blocksum(pe[:, 0:W], slice(128 * j, TC))
                        nc.tensor.matmul(po[:, 128 * j:], vnat[:, 1 + bb, :], pe[:, 0:W],
                                         start=False, stop=last)
                        if j % 2 == 1:
                            outproj_unit()
                    # normalize: oT[h][:, ts] = po * (1/den) broadcast
                    dsum = asb.tile([1, TC], F32, tag="dsum")
                    nc.vector.tensor_add(dsum[:], denp[0][:], denp[1][:])
                    rc = asb.tile([1, TC], BF16, tag="rc")
                    with nc.allow_low_precision(reason="softmax denom in bf16"):
                        nc.vector.reciprocal(rc[:], dsum[:])
                    rb = asb.tile([128, TC], BF16, tag="rb")
                    nc.gpsimd.partition_broadcast(rb[:], rc[:], channels=128)
                    nc.vector.tensor_mul(oT[h][:, ts], po[:], rb[:])
                    while h == NQH - 1 and outproj_unit():
                        pass

                if c == NTC - 1:
                    op_state = {"u": 0, "ob": None, "base": 4 * c}
                    while outproj_unit():
                        pass


    nc.compile()
    return nc


_NC_CACHE = {}


def _get_nc():
    if "nc" not in _NC_CACHE:
        _NC_CACHE["nc"] = build_nc()
    return _NC_CACHE["nc"]


def _bf(x):
    return np.ascontiguousarray(x.astype(ml_dtypes.bfloat16))


def kernel(**inputs) -> np.ndarray:
    f = lambda k: np.asarray(inputs[k], np.float32)
    hs = f("hidden_states")[0]            # (T, D)
    vk = f("virtual_keys")[0]             # (HKV, R, HD)
    vv = f("virtual_values")[0]
    Wq, Wk, Wv, Wo = f("Wq"), f("Wk"), f("Wv"), f("Wo")
    qnw, knw = f("q_norm_w"), f("k_norm_w")
    lkA, lkB = f("lora_k_A"), f("lora_k_B")
    lvA, lvB = f("lora_v_A"), f("lora_v_B")
    sk = np.float32(np.asarray(inputs["scale_k"]))
    sv = np.float32(np.asarray(inputs["scale_v"]))
    cos, sin = f("cos"), f("sin")         # (T, HD)

    # packed tiles: hsp[c*4+i][p][d8*TC+t] = hs[c*TC+t, (8i+d8)*128+p]
    hsT32 = hs.T.reshape(ND, 128, NTC, TC)          # [dtile, p, c, t]
    hsp = _bf(hsT32.transpose(2, 0, 1, 3)           # [c, dtile, p, t]
              .reshape(NTC, 4, 8, 128, TC)
              .transpose(0, 1, 3, 2, 4)
              .reshape(NTC * 4, 128, 8 * TC))
    # weighted cos/sin for fused (rms*w) + rope:
    #   cw[d,t] = w[d]*cos[t,d]
    #   sw[d,t] = -w[d+64]*sin[t,d]  (d<64);  w[d-64]*sin[t,d]  (d>=64)
    def cw_sw(w):
        cw = (cos.T * w[:, None]).astype(np.float32)
        sw = np.empty((HD, T), np.float32)
        sw[0:64] = -w[64:128, None] * sin.T[0:64]
        sw[64:128] = w[0:64, None] * sin.T[64:128]
        return _bf(cw), _bf(sw)
    cwqh, swqh = cw_sw(qnw)
    cwkh, swkh = cw_sw(knw)
    # host-side LoRA adaptation of the virtual KV (tiny)
    vk_a = vk + sk * (vk @ lkA @ lkB)      # (HKV, R, HD)
    vv_a = vv + sv * (vv @ lvA @ lvB)
    # constant [128,128] triangular mask: allowed k<=q, else -1e30
    idx = np.arange(128)
    masktri = np.where(idx[:, None] <= idx[None, :], 0.0, -1e30).astype(np.float32)

    def wpp_m(m):
        cols = [Wq[:, 512 * m + 128 * p:512 * m + 128 * (p + 1)] for p in range(4)]
        cols.append(Wk[:, 128 * m:128 * (m + 1)])
        cols.append(Wv[:, 128 * m:128 * (m + 1)])
        blocks = [c.reshape(ND, 128, 128).transpose(1, 0, 2).reshape(128, ND * 128)
                  for c in cols]
        return _bf(np.stack(blocks, axis=0))

    in_maps = []
    for m in range(8):
        in_maps.append({
            "hsp": hsp,
            "wpp": wpp_m(m),
            "wo": _bf(Wo[512 * m:512 * (m + 1), :]),
            "cwq": cwqh, "swq": swqh, "cwk": cwkh, "swk": swkh,
            "masktri": masktri,
            "kvirt": _bf(vk_a[m].T),
            "vvirt": _bf(vv_a[m]),
        })

    nc = _get_nc()
    res = run_bass_kernel_spmd(nc, in_maps, core_ids=list(range(8)))
    acc = res.results[0]["out"].astype(np.float32)
    for m in range(1, 8):
        acc = acc + res.results[m]["out"].astype(np.float32)
    return acc[None]  # (1, T, D)


# revision 20
# speedup vs baseline: 1.2555x; 1.0111x over previous
"""Trainium2 Bass kernel for KVAdapterInjector (Qwen3-style GQA attention with
LoRA-adapted virtual KV prefix).

Sharding: tensor-parallel over heads across 8 cores. Core m gets KV head m and
Q heads 4m..4m+3. Wq/Wk/Wv sharded on output dim, Wo on input dim; partial
outputs (bf16) summed on host.

v2 design notes (cost-model driven):
- All heavy matmuls in bf16 (1.0 cycles/row, immune to the fp32r ap<256
  penalty). PSUM accumulation stays fp32. Measured end-to-end bf16 error
  ~5.5e-3 (budget 2e-2). fp8 was measured at 2.7-5e-2 per stage: rejected.
- PE-row accounting puts the tensor engine at ~370us; all other engines are
  kept under ~150us: softmax denominators stay as ones-matmuls on PE, but
  rms-norm sum/broadcast use gpsimd partition_all_reduce/broadcast (Pool),
  rsqrt = exp(-0.5*ln(x)) on Act (single activation table: ln+exp+square),
  mask-adds and PSUM drains ride Pool, rope elementwise rides DVE in bf16
  (2x mode).
- Causal diagonal blocks are trimmed: block j of a 512-query chunk only
  computes queries >= 128*j, with a constant [128,128] triangular mask tile.
- Chunk-pipelined: proj(c) -> norm/rope(c) -> attention(c) -> outproj(c),
  with PSUM pools sized to exactly 8 banks so phases from adjacent chunks
  overlap across engines.
"""
import sys

sys.path.insert(0, "/opt/trn_rl_repo")

import numpy as np
import ml_dtypes

import concourse.bass as bass
import concourse.mybir as mybir
import concourse.tile as tile
from concourse import bacc
from concourse import bass_isa
from concourse.bass_utils import run_bass_kernel_spmd

F32 = mybir.dt.float32
F32R = mybir.dt.float32r
BF16 = mybir.dt.bfloat16
AX = mybir.AxisListType
ALU = mybir.AluOpType
ACTF = mybir.ActivationFunctionType
RED = bass_isa.ReduceOp

T = 2048
D = 4096
HD = 128
NQH = 4          # q heads per core
R = 64           # virtual tokens
RANK = 16
EPS = 1e-6
SCALING = HD ** -0.5
NTC = 4          # T chunks of 512
TC = 512
ND = D // 128    # 32 contraction tiles
NKB = T // 128   # 16 key blocks (real)


def build_nc():
    nc = bacc.Bacc(None, target_bir_lowering=False, debug=False)

    # ---- DRAM I/O (bf16 activations/weights prepared on host) ----
    wpp = nc.dram_tensor("wpp", (6, 128, ND * 128), BF16, kind="ExternalInput")
    hsp = nc.dram_tensor("hsp", (NTC * 4, 128, 8 * TC), BF16, kind="ExternalInput")
    wo = nc.dram_tensor("wo", (NQH * HD, D), BF16, kind="ExternalInput")
    cwq = nc.dram_tensor("cwq", (HD, T), BF16, kind="ExternalInput")
    swq = nc.dram_tensor("swq", (HD, T), BF16, kind="ExternalInput")
    cwk = nc.dram_tensor("cwk", (HD, T), BF16, kind="ExternalInput")
    swk = nc.dram_tensor("swk", (HD, T), BF16, kind="ExternalInput")
    masktri = nc.dram_tensor("masktri", (128, 128), F32, kind="ExternalInput")
    kvirt = nc.dram_tensor("kvirt", (HD, R), BF16, kind="ExternalInput")
    vvirt = nc.dram_tensor("vvirt", (R, HD), BF16, kind="ExternalInput")
    out = nc.dram_tensor("out", (T, D), BF16, kind="ExternalOutput")

    from contextlib import ExitStack
    with tile.TileContext(nc) as tc, ExitStack() as est:
        cp = est.enter_context(tc.tile_pool(name="consts", bufs=1))
        pp = est.enter_context(tc.tile_pool(name="persist", bufs=1))

        # pin the Act table that serves square+ln+exp, so the auto-insertion
        # pass doesn't thrash between natural_log and exp tables
        from concourse.hw_specs import get_activation_tables
        _tables = list(get_activation_tables(nc.m.arch).keys())
        _atl = mybir.InstLoadActFuncSet(
            name=nc.get_next_instruction_name(), ins=[], outs=[],
            act_func_set_id=_tables.index("natural_log_exp_and_others"))
        _atl.engine = mybir.EngineType.Activation
        nc.scalar.add_instruction(_atl)

        # ---- small consts ----
        onesb = cp.tile([128, 1], BF16)
        nc.vector.memset(onesb[:], 1.0)
        epsc = cp.tile([128, 1], F32)
        nc.vector.memset(epsc[:], EPS)
        zeroc = cp.tile([128, 1], F32)
        nc.vector.memset(zeroc[:], 0.0)
        mask_s = cp.tile([128, 128], F32)

        # ---- persistent activations ----
        # qT[h]: rope'd queries, [HD, T] bf16; aliased as oT (attention output)
        qT = [pp.tile([HD, T], BF16, tag=f"qT{h}", name=f"qT{h}") for h in range(NQH)]
        oT = qT
        kT = pp.tile([HD, R + T], BF16)           # cols 0:64 = adapted virtual keys
        vnat = pp.tile([128, NKB + 1, 128], BF16)  # block 0 = virtual values (rows 0:64)

        # ---- rope/norm consts (weighted cos/sin) ----
        cwq_s = cp.tile([HD, T], BF16)
        swq_s = cp.tile([HD, T], BF16)
        cwk_s = cp.tile([HD, T], BF16)
        swk_s = cp.tile([HD, T], BF16)

        # ---- weights in SBUF ----
        wqkv_s = cp.tile([128, 6, ND, 128], BF16)  # passes q0..q3, k, v
        wo_s = cp.tile([128, NQH, D], BF16)

        # ---- LoRA-adapted virtual KV is computed on host (2 MFLOP) ----

        # ---- weight / rope-const loads, ordered for earliest PE start ----
        pm = lambda ap: ap.rearrange("(n p) c -> p n c", p=128)
        nc.sync.dma_start(wqkv_s[:, 0, :, :], wpp[0])

        # ================= main chunk pipeline =================
        with tc.tile_pool(name="proj_ps", bufs=2, space="PSUM") as prps, \
             tc.tile_pool(name="mm_ps", bufs=4, space="PSUM") as mmps, \
             tc.tile_pool(name="b2k_ps", bufs=2, space="PSUM") as b2ps, \
             tc.tile_pool(name="hs_sb", bufs=1) as hsb, \
             tc.tile_pool(name="nrm_sb", bufs=2) as nsb, \
             tc.tile_pool(name="pe_sb", bufs=7) as peb, \
             tc.tile_pool(name="at_sb", bufs=2) as asb, \
             tc.tile_pool(name="ob_sb", bufs=2) as obb:
            def new_hs(c_):
                return [hsb.tile([128, 8, TC], BF16, tag=f"hs{i}",
                                 name=f"hs{c_}_{i}") for i in range(4)]
            def load_hs(tiles_, c_, engs=None):
                for i in range(4):
                    eng = nc.sync if engs is None else engs[i]
                    eng.dma_start(tiles_[i][:], hsp[4 * c_ + i])
            hs_tiles = {0: new_hs(0)}
            load_hs(hs_tiles[0], 0, engs=[nc.scalar, nc.sync, nc.scalar, nc.sync])
            nc.scalar.dma_start(mask_s[:], masktri[:])
            nc.scalar.dma_start(kT[:, 0:R], kvirt[:])
            nc.scalar.dma_start(vnat[0:R, 0, :], vvirt[:])
            nc.sync.dma_start(wqkv_s[:, 1, :, :], wpp[1])
            nc.sync.dma_start(wqkv_s[:, 2, :, :], wpp[2])
            nc.sync.dma_start(cwq_s[:], cwq[:])
            nc.sync.dma_start(swq_s[:], swq[:])
            nc.sync.dma_start(wqkv_s[:, 3, :, :], wpp[3])
            nc.sync.dma_start(wqkv_s[:, 4, :, :], wpp[4])
            nc.sync.dma_start(wqkv_s[:, 5, :, :], wpp[5])
            nc.sync.dma_start(cwk_s[:], cwk[:])
            nc.sync.dma_start(swk_s[:], swk[:])
            nc.sync.dma_start(wo_s[:], pm(wo[:, :]))
            for c in range(NTC):
                ts = slice(c * TC, (c + 1) * TC)
                hs_c = hs_tiles.pop(c)

                # ---- projections: 5 passes (q0..q3, k), each one accumulator ----
                for p in range(NQH + 1):
                    pacc = prps.tile([128, TC], F32, tag="pacc")
                    wslice = wqkv_s[:, p, :, :]
                    for d in range(ND):
                        nc.tensor.matmul(pacc[:], wslice[:, d, :],
                                         hs_c[d // 8][:, d % 8, :],
                                         start=(d == 0), stop=(d == ND - 1))
                    # ---- rms-norm + rope on this pass's PSUM ----
                    isq = p < NQH
                    cw = cwq_s if isq else cwk_s
                    sw = swq_s if isq else swk_s
                    dst = qT[p][:, ts] if isq else kT[:, R + c * TC: R + (c + 1) * TC]
                    sq = nsb.tile([HD, TC], BF16, tag="sq")
                    nc.gpsimd.tensor_mul(sq[:], pacc[:], pacc[:])
                    ssum = nsb.tile([HD, TC], BF16, tag="ssum")
                    nc.gpsimd.partition_all_reduce(ssum[:], sq[:], channels=128,
                                                   reduce_op=RED.add)
                    lns = nsb.tile([HD, TC], F32, tag="lns")
                    nc.scalar.activation(lns[:], ssum[:], ACTF.Ln,
                                         scale=1.0 / HD, bias=epsc[:])
                    rinv = nsb.tile([HD, TC], BF16, tag="rinv")
                    nc.scalar.activation(rinv[:], lns[:], ACTF.Exp, scale=-0.5,
                                         bias=zeroc[:])
                    xn = nsb.tile([HD, TC], BF16, tag="xn")
                    nc.vector.tensor_mul(xn[:], pacc[:], rinv[:])
                    t1 = nsb.tile([HD, TC], BF16, tag="t1")
                    nc.vector.tensor_mul(t1[:], xn[:], cw[:, ts])
                    t2 = nsb.tile([HD, TC], BF16, tag="t2")
                    nc.vector.tensor_mul(t2[0:64, :], xn[64:128, :], sw[0:64, ts])
                    nc.vector.tensor_mul(t2[64:128, :], xn[0:64, :], sw[64:128, ts])
                    nc.vector.tensor_add(dst, t1[:], t2[:])

                # ---- V in natural layout: stationary = hs t-slices ----
                vacc = b2ps.tile([128, 4, 128], F32, tag="b2k")
                for tt in range(4):
                    for d in range(ND):
                        nc.tensor.matmul(vacc[:, tt, :],
                                         hs_c[d // 8][:, d % 8,
                                                      tt * 128:(tt + 1) * 128],
                                         wqkv_s[:, 5, d, :],
                                         start=(d == 0), stop=(d == ND - 1))
                nc.gpsimd.tensor_copy(vnat[:, 1 + 4 * c: 5 + 4 * c, :], vacc[:])

                # prefetch next chunk's hidden states (after last hs_c reader issued)
                if c + 1 < NTC:
                    hs_tiles[c + 1] = new_hs(c + 1)
                    load_hs(hs_tiles[c + 1], c + 1)

                # ---- attention for this chunk's queries, interleaved with
                # ---- the previous chunk's output projection (fills PE while
                # ---- attention is Act-bound)
                op_state = {"u": 0, "ob": None, "base": 4 * (c - 1)}

                def outproj_unit():
                    # emit one (tt, j2) unit of the previous chunk's output
                    # projection; group 4 units per ob tile + one DMA
                    u = op_state["u"]
                    if u >= 32 * (1 if c > 0 else 0):
                        return False
                    op_state["u"] = u + 1
                    tt = op_state["base"] + u // 8
                    half = (u // 4) % 2
                    jj = u % 4
                    j2 = 4 * half + jj
                    if jj == 0:
                        op_state["ob"] = obb.tile([128, 4, TC], BF16, tag="ob",
                                                  name=f"ob{c}_{u}")
                    ob = op_state["ob"]
                    po2 = mmps.tile([128, TC], F32, tag="mm")
                    for h2 in range(NQH):
                        nc.tensor.matmul(
                            po2[:], oT[h2][:, tt * 128:(tt + 1) * 128],
                            wo_s[:, h2, j2 * TC:(j2 + 1) * TC],
                            start=(h2 == 0), stop=(h2 == NQH - 1))
                    final = op_state.get("final") and u >= 24
                    if final and u % 3 == 2:
                        nc.scalar.copy(ob[:, jj, :], po2[:])
                    else:
                        if final:
                            dr = nc.gpsimd if u % 3 == 0 else nc.vector
                        else:
                            dr = nc.gpsimd if j2 % 2 == 0 else nc.vector
                        dr.tensor_copy(ob[:, jj, :], po2[:])
                    cols = slice(half * 2048 + 512 * jj - 512, half * 2048 + 512 * jj + 512)
                    if final and jj % 2 == 1:
                        eng = nc.sync if jj == 1 else nc.scalar
                        eng.dma_start(out[tt * 128:(tt + 1) * 128, cols],
                                      ob[:, jj - 1:jj + 1, :])
                    elif (not final) and jj == 3:
                        eng = nc.sync if tt % 2 == 0 else nc.scalar
                        eng.dma_start(
                            out[tt * 128:(tt + 1) * 128,
                                half * 2048:(half + 1) * 2048], ob[:])
                    return True

                for h in range(NQH):
                    # two parity accumulators halve the serial add chain
                    denp = [asb.tile([1, TC], F32, tag="denE", bufs=2, name="denE"),
                            asb.tile([1, TC], F32, tag="denO", bufs=2, name="denO")]
                    blk_i = [0]
                    po = b2ps.tile([128, TC], F32, tag="b2k")

                    def blocksum(pe_ap, wslc):
                        # denominator accumulation off-PE: partition-reduce
                        # on Pool, then row-add into den parity acc on DVE
                        i = blk_i[0]; blk_i[0] += 1
                        den_ = denp[i % 2]
                        ps_ = peb.tile([128, TC], BF16, tag="ps", bufs=4)
                        rows = pe_ap.shape[0]
                        W_ = pe_ap.shape[-1]
                        nc.gpsimd.partition_all_reduce(
                            ps_[0:rows, 0:W_], pe_ap, channels=rows,
                            reduce_op=RED.add)
                        if i < 2:
                            nc.vector.tensor_copy(den_[:], ps_[0:1, 0:W_])
                        else:
                            nc.vector.tensor_add(den_[:, wslc], den_[:, wslc],
                                                 ps_[0:1, 0:W_])

                    # virtual block (full width)
                    st_ = mmps.tile([128, TC], F32, tag="mm")
                    nc.tensor.matmul(st_[0:R, :], kT[:, 0:R], qT[h][:, ts],
                                     start=True, stop=True)
                    pe = peb.tile([128, TC], BF16, tag="pe")
                    nc.scalar.activation(pe[0:R, :], st_[0:R, :], ACTF.Exp,
                                         scale=SCALING, bias=zeroc[0:R, :])
                    blocksum(pe[0:R, :], slice(0, TC))
                    nc.tensor.matmul(po[:], vnat[0:R, 0, :], pe[0:R, :],
                                     start=True, stop=False)
                    # full (past) key blocks
                    for bb in range(4 * c):
                        st_ = mmps.tile([128, TC], F32, tag="mm")
                        nc.tensor.matmul(st_[:], kT[:, R + bb * 128: R + (bb + 1) * 128],
                                         qT[h][:, ts], start=True, stop=True)
                        pe = peb.tile([128, TC], BF16, tag="pe")
                        nc.scalar.activation(pe[:], st_[:], ACTF.Exp,
                                             scale=SCALING, bias=zeroc[:])
                        blocksum(pe[:], slice(0, TC))
                        nc.tensor.matmul(po[:], vnat[:, 1 + bb, :], pe[:],
                                         start=False, stop=False)
                        if bb % 2 == 1:
                            outproj_unit()
                    # diagonal blocks j=0..3: queries >= 128*j only
                    for j in range(4):
                        bb = 4 * c + j
                        W = TC - 128 * j
                        qs = slice(c * TC + 128 * j, (c + 1) * TC)
                        st_ = mmps.tile([128, TC], F32, tag="mm")
                        nc.tensor.matmul(st_[:, 0:W],
                                         kT[:, R + bb * 128: R + (bb + 1) * 128],
                                         qT[h][:, qs], start=True, stop=True)
                        # triangular mask on the first 128 cols of this region
                        nc.gpsimd.tensor_add(st_[:, 0:128], st_[:, 0:128], mask_s[:])
                        pe = peb.tile([128, TC], BF16, tag="pe")
                        nc.scalar.activation(pe[:, 0:W], st_[:, 0:W], ACTF.Exp,
                                             scale=SCALING, bias=zeroc[:])
                        last = (j == 3)
                        blocksum(pe[:, 0:W], slice(128 * j, TC))
                        nc.tensor.matmul(po[:, 128 * j:], vnat[:, 1 + bb, :], pe[:, 0:W],
                                         start=False, stop=last)
                        if j % 2 == 1:
                            outproj_unit()
                    # normalize: oT[h][:, ts] = po * (1/den) broadcast
                    dsum = asb.tile([1, TC], F32, tag="dsum")
                    nc.vector.tensor_add(dsum[:], denp[0][:], denp[1][:])
                    rc = asb.tile([1, TC], BF16, tag="rc")
                    with nc.allow_low_precision(reason="softmax denom in bf16"):
                        nc.vector.reciprocal(rc[:], dsum[:])
                    rb = asb.tile([128, TC], BF16, tag="rb")
                    nc.gpsimd.partition_broadcast(rb[:], rc[:], channels=128)
                    nc.vector.tensor_mul(oT[h][:, ts], po[:], rb[:])
                    while h == NQH - 1 and outproj_unit():
                        pass

                if c == NTC - 1:
                    op_state = {"u": 0, "ob": None, "base": 4 * c, "final": True}
                    while outproj_unit():
                        pass


    nc.compile()
    return nc


_NC_CACHE = {}


def _get_nc():
    if "nc" not in _NC_CACHE:
        _NC_CACHE["nc"] = build_nc()
    return _NC_CACHE["nc"]


def _bf(x):
    return np.ascontiguousarray(x.astype(ml_dtypes.bfloat16))


def kernel(**inputs) -> np.ndarray:
    f = lambda k: np.asarray(inputs[k], np.float32)
    hs = f("hidden_states")[0]            # (T, D)
    vk = f("virtual_keys")[0]             # (HKV, R, HD)
    vv = f("virtual_values")[0]
    Wq, Wk, Wv, Wo = f("Wq"), f("Wk"), f("Wv"), f("Wo")
    qnw, knw = f("q_norm_w"), f("k_norm_w")
    lkA, lkB = f("lora_k_A"), f("lora_k_B")
    lvA, lvB = f("lora_v_A"), f("lora_v_B")
    sk = np.float32(np.asarray(inputs["scale_k"]))
    sv = np.float32(np.asarray(inputs["scale_v"]))
    cos, sin = f("cos"), f("sin")         # (T, HD)

    # packed tiles: hsp[c*4+i][p][d8*TC+t] = hs[c*TC+t, (8i+d8)*128+p]
    hsT32 = hs.T.reshape(ND, 128, NTC, TC)          # [dtile, p, c, t]
    hsp = _bf(hsT32.transpose(2, 0, 1, 3)           # [c, dtile, p, t]
              .reshape(NTC, 4, 8, 128, TC)
              .transpose(0, 1, 3, 2, 4)
              .reshape(NTC * 4, 128, 8 * TC))
    # weighted cos/sin for fused (rms*w) + rope:
    #   cw[d,t] = w[d]*cos[t,d]
    #   sw[d,t] = -w[d+64]*sin[t,d]  (d<64);  w[d-64]*sin[t,d]  (d>=64)
    def cw_sw(w):
        cw = (cos.T * w[:, None]).astype(np.float32)
        sw = np.empty((HD, T), np.float32)
        sw[0:64] = -w[64:128, None] * sin.T[0:64]
        sw[64:128] = w[0:64, None] * sin.T[64:128]
        return _bf(cw), _bf(sw)
    cwqh, swqh = cw_sw(qnw)
    cwkh, swkh = cw_sw(knw)
    # host-side LoRA adaptation of the virtual KV (tiny)
    vk_a = vk + sk * (vk @ lkA @ lkB)      # (HKV, R, HD)
    vv_a = vv + sv * (vv @ lvA @ lvB)
    # constant [128,128] triangular mask: allowed k<=q, else -1e30
    idx = np.arange(128)
    masktri = np.where(idx[:, None] <= idx[None, :], 0.0, -1e30).astype(np.float32)

    def wpp_m(m):
        cols = [Wq[:, 512 * m + 128 * p:512 * m + 128 * (p + 1)] for p in range(4)]
        cols.append(Wk[:, 128 * m:128 * (m + 1)])
        cols.append(Wv[:, 128 * m:128 * (m + 1)])
        blocks = [c.reshape(ND, 128, 128).transpose(1, 0, 2).reshape(128, ND * 128)
                  for c in cols]
        return _bf(np.stack(blocks, axis=0))

    in_maps = []
    for m in range(8):
        in_maps.append({
            "hsp": hsp,
            "wpp": wpp_m(m),
            "wo": _bf(Wo[512 * m:512 * (m + 1), :]),
            "cwq": cwqh, "swq": swqh, "cwk": cwkh, "swk": swkh,
            "masktri": masktri,
            "kvirt": _bf(vk_a[m].T),
            "vvirt": _bf(vv_a[m]),
        })

    nc = _get_nc()
    res = run_bass_kernel_spmd(nc, in_maps, core_ids=list(range(8)))
    acc = res.results[0]["out"].astype(np.float32)
    for m in range(1, 8):
        acc = acc + res.results[m]["out"].astype(np.float32)
    return acc[None]  # (1, T, D)
